# revision 1
# baseline (speedup 1.0000x reference)
"""Differential attention kernel for 8 Trainium2 NeuronCores — v3.

- v1 attention structure (per-skt scores + 512-wide exp; pairing hurt).
- merged projection pass (q chunks fused, xT read once).
- per-tag psum bufs: score/proj accumulators get 4 slots, v 2, pv 4.
- weight DMAs split per d-tile, wk issued first, so the first k-projection
  matmul starts ~1us in instead of waiting 12.5us for bulk weight DMA.
"""

import math
import os
import time
from contextlib import ExitStack

import ml_dtypes
import numpy as np

import concourse.bass as bass
from concourse import bacc
import concourse.mybir as mybir
import concourse.tile as tile
from concourse.bass_utils import run_bass_kernel_spmd

B, S, D = 4, 4096, 2048
HD = 128
DV = 256
DVA = DV + 1      # + ones column for row sums
SQ = S // 2
N_CORES = 8
DEPTH = 12
SCALE = HD ** -0.5

DT_P = D // 128   # 16 d-tiles
SKT = S // 128    # 32 key tiles
SC = S // 512     # 8 s-chunks
QC = SQ // 512    # 4 q-chunks
SQT = SQ // 128   # 16 q tiles

BF16 = mybir.dt.bfloat16
F32 = mybir.dt.float32

_cache = {}


def build_nc():
    nc = bacc.Bacc("TRN2", target_bir_lowering=False, debug=False)

    xT_d = nc.declare_dram_parameter("xT", [D, S], BF16, isOutput=False)
    wq_d = nc.declare_dram_parameter("wq", [D, DV], BF16, isOutput=False)
    wk_d = nc.declare_dram_parameter("wk", [D, DV], BF16, isOutput=False)
    wv_d = nc.declare_dram_parameter("wv", [D, DV], BF16, isOutput=False)
    lam_d = nc.declare_dram_parameter("lam", [128, 1], F32, isOutput=False)
    out_d = nc.declare_dram_parameter("out", [SQ, DV], F32, isOutput=True)

    xT = xT_d.ap()
    out = out_d.ap()

    with tile.TileContext(nc) as tc, ExitStack() as ctx:
        singles = ctx.enter_context(tc.tile_pool(name="singles", bufs=1))
        x_pool = ctx.enter_context(tc.tile_pool(name="x", bufs=40))
        e_pool = ctx.enter_context(tc.tile_pool(name="e", bufs=6))
        o_pool = ctx.enter_context(tc.tile_pool(name="o", bufs=4))
        r_pool = ctx.enter_context(tc.tile_pool(name="r", bufs=8))

        # --- resident SBUF tensors; weights DMA'd per d-tile, wk first --
        w_sb = {n: singles.tile([128, DT_P, DV], BF16, tag=f"w_{n}", name=f"w_{n}")
                for n in ("wk", "wq", "wv")}
        w_ap = {"wk": wk_d.ap(), "wq": wq_d.ap(), "wv": wv_d.ap()}
        lam_sb = singles.tile([128, 1], F32, tag="lam")
        nc.sync.dma_start(out=lam_sb, in_=lam_d.ap())

        # chunk-0 x tiles interleaved with wk so the first k matmul can
        # start ~1us in; wv/wq follow (needed later in chunk 0)
        xts0 = []
        for dt in range(DT_P):
            nc.sync.dma_start(
                out=w_sb["wk"][:, dt, :],
                in_=w_ap["wk"][dt * 128:(dt + 1) * 128, :],
            )
            xt = x_pool.tile([128, 512], BF16, tag="xt", name=f"xt0_{dt}")
            nc.sync.dma_start(out=xt, in_=xT[dt * 128:(dt + 1) * 128, 0:512])
            xts0.append(xt)
        for n in ("wv", "wq"):
            for dt in range(DT_P):
                nc.sync.dma_start(
                    out=w_sb[n][:, dt, :],
                    in_=w_ap[n][dt * 128:(dt + 1) * 128, :],
                )

        kT = singles.tile([128, 2, S], BF16, tag="kT")        # [dh, head, sk]
        qT = singles.tile([128, 2, SQ], BF16, tag="qT")       # [dh, head, sq]
        v_aug = singles.tile([128, SKT, DVA], BF16, tag="v")  # [s_row, s_tile, dv+1]
        pv1 = singles.tile([128, SQT, DVA], F32, tag="pv1")   # head-1 PV staging

        nc.vector.memset(v_aug[:, :, DV:DVA], 1.0)            # ones column

        # --- projections: one pass over the 8 s-chunks ------------------
        pctx = ExitStack()
        psum = pctx.enter_context(
            tc.tile_pool(name="psum_proj", bufs=4, space=bass.MemorySpace.PSUM)
        )

        # PE warm-up: junk matmuls fill the initial DMA wait so HAM is at
        # 2.4GHz when the first real projection matmul issues.
        jt = singles.tile([128, 512], BF16, tag="junk")
        nc.vector.memset(jt, 0.0)
        jps = psum.tile([128, 512], F32, tag="big_ps", bufs=4, name="jps")
        for w in range(48):
            nc.tensor.matmul(jps, jt[:, 0:128], jt, start=True, stop=True)
        nc.vector.tensor_copy(jt, jps)
        for sc in range(SC):
            if sc == 0:
                xts = xts0
            else:
                xts = []
                for dt in range(DT_P):
                    xt = x_pool.tile([128, 512], BF16, tag="xt", name=f"xt{sc}_{dt}")
                    nc.sync.dma_start(
                        out=xt,
                        in_=xT[dt * 128:(dt + 1) * 128, sc * 512:(sc + 1) * 512],
                    )
                    xts.append(xt)

            projs = [("wk", kT)] + ([("wq", qT)] if sc < QC else [])
            for wname, dst in projs:
                for h in range(2):
                    ps = psum.tile([128, 512], F32, tag="big_ps", bufs=4,
                                   name=f"ps{sc}{wname}{h}")
                    for dt in range(DT_P):
                        nc.tensor.matmul(
                            ps,
                            w_sb[wname][:, dt, h * HD:(h + 1) * HD],
                            xts[dt],
                            start=(dt == 0),
                            stop=(dt == DT_P - 1),
                        )
                    nc.vector.tensor_copy(dst[:, h, sc * 512:(sc + 1) * 512], ps)
            for i in range(4):
                vps = psum.tile([128, DV], F32, tag="v_ps", bufs=2,
                                name=f"vps{sc}_{i}")
                for dt in range(DT_P):
                    nc.tensor.matmul(
                        vps,
                        xts[dt][:, i * 128:(i + 1) * 128],
                        w_sb["wv"][:, dt, :],
                        start=(dt == 0),
                        stop=(dt == DT_P - 1),
                    )
                nc.vector.tensor_copy(v_aug[:, sc * 4 + i, 0:DV], vps)

        pctx.close()

        # --- attention: head 1 then head 2 ------------------------------
        psum = ctx.enter_context(
            tc.tile_pool(name="psum_att", bufs=4, space=bass.MemorySpace.PSUM)
        )
        psum_pv = ctx.enter_context(
            tc.tile_pool(name="psum_pv", bufs=4, space=bass.MemorySpace.PSUM)
        )
        for h in range(2):
            for qc in range(QC):
                pv_ps = [
                    psum_pv.tile([128, DVA], F32, tag="pv_ps", name=f"pv_ps{i}")
                    for i in range(4)
                ]
                for skt in range(SKT):
                    sps = psum.tile([128, 512], F32, tag="big_ps", bufs=4,
                                    name=f"sps{skt}")
                    nc.tensor.matmul(
                        sps,
                        kT[:, h, skt * 128:(skt + 1) * 128],
                        qT[:, h, qc * 512:(qc + 1) * 512],
                    )
                    et = e_pool.tile([128, 512], BF16, tag="et", name=f"et{skt}")
                    nc.scalar.activation(
                        out=et, in_=sps,
                        func=mybir.ActivationFunctionType.Exp,
                        scale=SCALE,
                    )
                    for i in range(4):
                        nc.tensor.matmul(
                            pv_ps[i],
                            et[:, i * 128:(i + 1) * 128],
                            v_aug[:, skt, :],
                            start=(skt == 0),
                            stop=(skt == SKT - 1),
                        )
                for i in range(4):
                    idx = qc * 4 + i
                    if h == 0:
                        nc.vector.tensor_copy(pv1[:, idx, :], pv_ps[i])
                    else:
                        r1 = r_pool.tile([128, 1], F32, tag="r1", name=f"r1_{idx}")
                        r2 = r_pool.tile([128, 1], F32, tag="r2", name=f"r2_{idx}")
                        nc.vector.reciprocal(r1, pv1[:, idx, DV:DVA])
                        nc.vector.reciprocal(r2, pv_ps[i][:, DV:DVA])
                        r2l = r_pool.tile([128, 1], F32, tag="r2l", name=f"r2l_{idx}")
                        nc.vector.tensor_mul(r2l, r2, lam_sb)
                        o1 = o_pool.tile([128, DV], F32, tag="o1", name=f"o1_{idx}")
                        o2 = o_pool.tile([128, DV], F32, tag="o2", name=f"o2_{idx}")
                        nc.vector.tensor_scalar_mul(o1, pv1[:, idx, 0:DV], r1)
                        nc.vector.tensor_scalar_mul(o2, pv_ps[i][:, 0:DV], r2l)
                        ot = o_pool.tile([128, DV], F32, tag="ot", name=f"ot_{idx}")
                        nc.vector.tensor_sub(ot, o1, o2)
                        nc.sync.dma_start(
                            out=out[idx * 128:(idx + 1) * 128, :], in_=ot
                        )

    nc.compile()
    return nc


def _lam(lambda_q1, lambda_q2, lambda_k1, lambda_k2):
    lam_init = 0.8 - 0.6 * math.exp(-0.3 * DEPTH)
    l1 = math.exp(float(np.sum(lambda_q1.astype(np.float64) * lambda_k1.astype(np.float64))))
    l2 = math.exp(float(np.sum(lambda_q2.astype(np.float64) * lambda_k2.astype(np.float64))))
    return l1 + l2 + lam_init


def kernel(x, WQ, WK, WV, lambda_q1, lambda_q2, lambda_k1, lambda_k2):
    if "nc" not in _cache:
        _cache["nc"] = build_nc()
    nc = _cache["nc"]

    bf = ml_dtypes.bfloat16
    lam = np.full((128, 1), _lam(lambda_q1, lambda_q2, lambda_k1, lambda_k2), np.float32)
    wq = np.ascontiguousarray(WQ, dtype=bf)
    wk = np.ascontiguousarray(WK, dtype=bf)
    wv = np.ascontiguousarray(WV, dtype=bf)

    in_maps = []
    for c in range(N_CORES):
        b, qs = c // 2, (c % 2) * SQ
        xb = x[b] if qs == 0 else np.concatenate([x[b, qs:], x[b, :qs]], axis=0)
        xT = np.ascontiguousarray(xb.T, dtype=bf)
        in_maps.append({"xT": xT, "wq": wq, "wk": wk, "wv": wv, "lam": lam})

    kres = None
    for attempt in range(3):
        try:
            kres = run_bass_kernel_spmd(nc, in_maps, list(range(N_CORES)))
            break
        except (ModuleNotFoundError, ImportError):
            # BASS_TRACE requested but this axon build has no NTFF hook
            os.environ["BASS_NEVER_TRACE"] = "1"
        except Exception:
            if attempt == 2:
                raise
            time.sleep(5)
    if kres is None:
        kres = run_bass_kernel_spmd(nc, in_maps, list(range(N_CORES)))
    _cache["last_results"] = kres
    res = kres.results

    out = np.empty((B, S, DV), np.float32)
    for c in range(N_CORES):
        b, qs = c // 2, (c % 2) * SQ
        out[b, qs:qs + SQ] = res[c]["out"]
    return out



# revision 2
# speedup vs baseline: 1.0062x; 1.0062x over previous
"""Differential attention kernel for 8 Trainium2 NeuronCores — v4.

Key-sharded: core c handles batch c//2 and key-half c%2 (2048 keys), all
4096 queries. Per-head partial numerators + row sums go back to the host,
which adds the two halves and normalizes (softmax denominators span both
cores' keys).

fp8 (e4m3) DoubleRow matmuls carry the projections and head-1 PV:
- projections use a 3-term hi/lo split (x and W each split into e4m3
  hi + residual lo; terms hi*hi + hi*lo + lo*hi) — bf16-level accuracy at
  0.75x the bf16 matmul cost.
- head-1 PV uses single-e4m3 exp scores against hi/lo-split V — half the
  bf16 PV cost. Head 2 (amplified by lambda≈2.8) stays bf16.
- scores stay bf16 (contraction is only 128, DoubleRow needs 256).
exp is biased by -3 so e4m3's max (224) covers the largest logit (~8.02).
"""

import math
import os
import time
from contextlib import ExitStack

import ml_dtypes
import numpy as np

import concourse.bass as bass
from concourse import bacc
import concourse.mybir as mybir
import concourse.tile as tile
from concourse.bass_utils import run_bass_kernel_spmd

B, S, D = 4, 4096, 2048
HD = 128
DV = 256
DVA = DV + 1      # + ones column for row sums
SK = S // 2       # keys per core
N_CORES = 8
DEPTH = 12
SCALE = HD ** -0.5

DT = D // 128     # 16 d-tiles
SKT = SK // 128   # 16 key tiles per core
KG = SKT // 2     # 8 key groups (DoubleRow pairs)
QC = S // 512     # 8 query chunks
KC = SK // 512    # 4 key chunks (phase-1 k/v projection)

W_SC = 32.0       # W pre-scale before e4m3 (W entries ~N(0, 1/D))
INV_SC = 1.0 / W_SC
ET_BIAS = -3.0    # exp(a - 3): keeps max exp < e4m3 max (224)

F8 = mybir.dt.float8e4
BF16 = mybir.dt.bfloat16
F32 = mybir.dt.float32
DR = mybir.MatmulPerfMode.DoubleRow
MULT = mybir.AluOpType.mult
SUB = mybir.AluOpType.subtract

_cache = {}


def build_nc():
    nc = bacc.Bacc("TRN2", target_bir_lowering=False, debug=False)

    xh_d = nc.declare_dram_parameter("xh", [D, S], F8, isOutput=False)
    xl_d = nc.declare_dram_parameter("xl", [D, S], F8, isOutput=False)
    wh_d = nc.declare_dram_parameter("wh", [D, 3 * DV], F8, isOutput=False)
    wl_d = nc.declare_dram_parameter("wl", [D, 3 * DV], F8, isOutput=False)
    out_d = nc.declare_dram_parameter("out", [2, S, DVA], BF16, isOutput=True)

    xh = xh_d.ap()
    xl = xl_d.ap()
    out = out_d.ap()

    with tile.TileContext(nc) as tc, ExitStack() as ctx:
        singles = ctx.enter_context(tc.tile_pool(name="singles", bufs=1))
        x_pool = ctx.enter_context(tc.tile_pool(name="x", bufs=6))
        e_pool = ctx.enter_context(tc.tile_pool(name="e", bufs=8))
        o_pool = ctx.enter_context(tc.tile_pool(name="o", bufs=8))

        w_hi = singles.tile([128, DT, 3 * DV], F8, tag="w_hi")
        w_lo = singles.tile([128, DT, 3 * DV], F8, tag="w_lo")
        kT = singles.tile([128, 2, SK], BF16, tag="kT")       # [dh, head, key]
        qT = singles.tile([128, 2, S], BF16, tag="qT")        # [dh, head, query]
        vhi = singles.tile([128, SKT, DVA], F8, tag="vhi")    # [key_row, kt, dv+1]
        vlo = singles.tile([128, SKT, DVA], F8, tag="vlo")
        vaug = singles.tile([128, SKT, DVA], BF16, tag="vaug")
        bias_t = singles.tile([128, 1], F32, tag="bias")

        nc.vector.memset(bias_t, ET_BIAS)
        nc.vector.memset(vhi[:, :, DV:DVA], 1.0)
        nc.vector.memset(vlo[:, :, DV:DVA], 0.0)
        nc.vector.memset(vaug[:, :, DV:DVA], 1.0)

        # weight DMAs per d-tile; first x chunk interleaved so the first
        # projection matmul can start early
        for dt in range(DT):
            nc.sync.dma_start(out=w_hi[:, dt, :], in_=wh_d.ap()[dt * 128:(dt + 1) * 128, :])
            nc.sync.dma_start(out=w_lo[:, dt, :], in_=wl_d.ap()[dt * 128:(dt + 1) * 128, :])

        def dma_chunk(sc, name):
            th = x_pool.tile([128, DT, 512], F8, tag="xt", name=f"xh_{name}")
            tl = x_pool.tile([128, DT, 512], F8, tag="xt", name=f"xl_{name}")
            for dt in range(DT):
                nc.sync.dma_start(out=th[:, dt, :], in_=xh[dt * 128:(dt + 1) * 128, sc * 512:(sc + 1) * 512])
                nc.sync.dma_start(out=tl[:, dt, :], in_=xl[dt * 128:(dt + 1) * 128, sc * 512:(sc + 1) * 512])
            return th, tl

        def proj3(ps, th, tl, col0, col1, xslice=None):
            """3-term hi/lo split projection: 24 DoubleRow matmuls into ps.

            If xslice is None: stationary = W cols [col0:col1], moving = x
            (output = [dv_cols, 512 rows]). Else stationary = x col slice,
            moving = W cols (output = [x rows, dv cols])."""
            terms = [(w_hi, th), (w_lo, th), (w_hi, tl)]
            n = len(terms)
            for t, (wt, xt) in enumerate(terms):
                for p in range(DT // 2):
                    first = (t == 0 and p == 0)
                    last = (t == n - 1 and p == DT // 2 - 1)
                    dpair = slice(2 * p, 2 * p + 2)
                    if xslice is None:
                        nc.tensor.matmul(ps, wt[:, dpair, col0:col1], xt[:, dpair, :],
                                         start=first, stop=last, perf_mode=DR)
                    else:
                        nc.tensor.matmul(ps, xt[:, dpair, xslice], wt[:, dpair, col0:col1],
                                         start=first, stop=last, perf_mode=DR)

        # ---- phase 1: k and v projections over this core's key half -----
        pctx = ExitStack()
        pp = pctx.enter_context(tc.tile_pool(name="pp", bufs=6, space=bass.MemorySpace.PSUM))

        # PE p-state warm-up while initial DMAs land
        jt = singles.tile([128, 512], BF16, tag="junk")
        nc.vector.memset(jt, 0.0)
        jps = pp.tile([128, 512], F32, tag="pp", name="jps")
        for _ in range(48):
            nc.tensor.matmul(jps, jt[:, 0:128], jt, start=True, stop=True)

        for sc in range(KC):
            th, tl = dma_chunk(sc, f"p1_{sc}")
            for h in range(2):
                ps = pp.tile([128, 512], F32, tag="pp", name=f"kps{sc}{h}")
                proj3(ps, th, tl, DV + h * HD, DV + (h + 1) * HD)
                nc.vector.tensor_scalar_mul(kT[:, h, sc * 512:(sc + 1) * 512], ps, INV_SC)
            for ssl in range(4):
                idx = sc * 4 + ssl
                vps = pp.tile([128, 512], F32, tag="pp", name=f"vps{idx}")
                proj3(vps[:, 0:DV], th, tl, 2 * DV, 3 * DV,
                      xslice=slice(ssl * 128, (ssl + 1) * 128))
                nc.vector.tensor_scalar_mul(vhi[:, idx, 0:DV], vps[:, 0:DV], INV_SC)
                nc.vector.scalar_tensor_tensor(vlo[:, idx, 0:DV], vps[:, 0:DV], INV_SC,
                                               vhi[:, idx, 0:DV], MULT, SUB)
                nc.vector.tensor_scalar_mul(vaug[:, idx, 0:DV], vps[:, 0:DV], INV_SC)
        pctx.close()

        # ---- phase 2: per query chunk: q proj, scores, exp, PV ----------
        ps_sc = ctx.enter_context(tc.tile_pool(name="ps_sc", bufs=2, space=bass.MemorySpace.PSUM))
        ps_pv = ctx.enter_context(tc.tile_pool(name="ps_pv", bufs=4, space=bass.MemorySpace.PSUM))

        for qc in range(QC):
            th, tl = dma_chunk(qc, f"p2_{qc}")
            qps = ps_sc.tile([128, 2, 512], F32, tag="sc", name=f"qps{qc}")
            for h in range(2):
                proj3(qps[:, h, :], th, tl, h * HD, (h + 1) * HD)
            for h in range(2):
                nc.vector.tensor_scalar_mul(qT[:, h, qc * 512:(qc + 1) * 512], qps[:, h, :], INV_SC)

            qTs = qT[:, 0, qc * 512:(qc + 1) * 512], qT[:, 1, qc * 512:(qc + 1) * 512]
            for h in range(2):
                pvs = [ps_pv.tile([128, DVA], F32, tag="pv", padded_shape=[128, 512],
                                  name=f"pv{qc}{h}{i}") for i in range(4)]
                for g in range(KG):
                    sg = ps_sc.tile([128, 2, 512], F32, tag="sc", name=f"sg{qc}{h}{g}")
                    for i in range(2):
                        kt = 2 * g + i
                        nc.tensor.matmul(sg[:, i, :], kT[:, h, kt * 128:(kt + 1) * 128],
                                         qTs[h], start=True, stop=True)
                    if h == 0:
                        e8 = e_pool.tile([128, 2, 512], F8, tag="e8", name=f"e8_{qc}{g}")
                        nc.scalar.activation(out=e8, in_=sg,
                                             func=mybir.ActivationFunctionType.Exp,
                                             bias=bias_t, scale=SCALE)
                        for qsl in range(4):
                            stat = e8[:, :, qsl * 128:(qsl + 1) * 128]
                            nc.tensor.matmul(pvs[qsl], stat, vhi[:, 2 * g:2 * g + 2, :],
                                             start=(g == 0), stop=False, perf_mode=DR)
                            nc.tensor.matmul(pvs[qsl], stat, vlo[:, 2 * g:2 * g + 2, :],
                                             start=False, stop=(g == KG - 1), perf_mode=DR)
                    else:
                        eb = e_pool.tile([128, 2, 512], BF16, tag="eb", name=f"eb_{qc}{g}")
                        nc.scalar.activation(out=eb, in_=sg,
                                             func=mybir.ActivationFunctionType.Exp,
                                             bias=bias_t, scale=SCALE)
                        for qsl in range(4):
                            for i in range(2):
                                nc.tensor.matmul(pvs[qsl], eb[:, i, qsl * 128:(qsl + 1) * 128],
                                                 vaug[:, 2 * g + i, :],
                                                 start=(g == 0 and i == 0),
                                                 stop=(g == KG - 1 and i == 1))
                for qsl in range(4):
                    onum = o_pool.tile([128, DVA], BF16, tag="onum", name=f"on{qc}{h}{qsl}")
                    nc.vector.tensor_copy(onum, pvs[qsl])
                    row = qc * 512 + qsl * 128
                    nc.sync.dma_start(out=out[h, row:row + 128, :], in_=onum)

    nc.compile()
    return nc


def _lam(lambda_q1, lambda_q2, lambda_k1, lambda_k2):
    lam_init = 0.8 - 0.6 * math.exp(-0.3 * DEPTH)
    l1 = math.exp(float(np.sum(lambda_q1.astype(np.float64) * lambda_k1.astype(np.float64))))
    l2 = math.exp(float(np.sum(lambda_q2.astype(np.float64) * lambda_k2.astype(np.float64))))
    return l1 + l2 + lam_init


def kernel(x, WQ, WK, WV, lambda_q1, lambda_q2, lambda_k1, lambda_k2):
    if "nc" not in _cache:
        _cache["nc"] = build_nc()
    nc = _cache["nc"]

    f8 = ml_dtypes.float8_e4m3
    lam = _lam(lambda_q1, lambda_q2, lambda_k1, lambda_k2)

    w = np.concatenate([np.asarray(WQ, np.float32), np.asarray(WK, np.float32),
                        np.asarray(WV, np.float32)], axis=1) * W_SC   # [D, 768]
    wh = np.ascontiguousarray(w, dtype=f8)
    wl = np.ascontiguousarray(w - wh.astype(np.float32), dtype=f8)

    in_maps = []
    for c in range(N_CORES):
        b, kh = c // 2, c % 2
        xb = np.asarray(x[b], np.float32)
        if kh:
            xb = np.concatenate([xb[SK:], xb[:SK]], axis=0)
        xbh = np.asarray(xb, f8)
        xbl = np.asarray(xb - xbh.astype(np.float32), f8)
        in_maps.append({
            "xh": np.ascontiguousarray(xbh.T),
            "xl": np.ascontiguousarray(xbl.T),
            "wh": wh, "wl": wl,
        })

    kres = None
    for attempt in range(3):
        try:
            kres = run_bass_kernel_spmd(nc, in_maps, list(range(N_CORES)))
            break
        except (ModuleNotFoundError, ImportError):
            os.environ["BASS_NEVER_TRACE"] = "1"
        except Exception:
            if attempt == 2:
                raise
            time.sleep(5)
    if kres is None:
        kres = run_bass_kernel_spmd(nc, in_maps, list(range(N_CORES)))
    _cache["last_results"] = kres
    _cache["input_names"] = ["xh", "xl", "wh", "wl"]
    res = kres.results

    out = np.empty((B, S, DV), np.float32)
    for b in range(B):
        na = res[2 * b]["out"].astype(np.float32)       # [2, S, DVA], canonical q
        nb = res[2 * b + 1]["out"].astype(np.float32)   # q rotated by SK
        nb = np.concatenate([nb[:, SK:, :], nb[:, :SK, :]], axis=1)
        n = na + nb
        o1 = n[0, :, :DV] / n[0, :, DV:DVA]
        o2 = n[1, :, :DV] / n[1, :, DV:DVA]
        out[b] = o1 - lam * o2
    return out


# revision 5
# speedup vs baseline: 1.1478x; 1.1408x over previous
"""Differential attention kernel for 8 Trainium2 NeuronCores — v4.

Key-sharded: core c handles batch c//2 and key-half c%2 (2048 keys), all
4096 queries. Per-head partial numerators + row sums go back to the host,
which adds the two halves and normalizes (softmax denominators span both
cores' keys).

fp8 (e4m3) DoubleRow matmuls carry the projections and head-1 PV:
- projections use a 3-term hi/lo split (x and W each split into e4m3
  hi + residual lo; terms hi*hi + hi*lo + lo*hi) — bf16-level accuracy at
  0.75x the bf16 matmul cost.
- head-1 PV uses single-e4m3 exp scores against hi/lo-split V — half the
  bf16 PV cost. Head 2 (amplified by lambda≈2.8) stays bf16.
- scores stay bf16 (contraction is only 128, DoubleRow needs 256).
exp is biased by -3 so e4m3's max (224) covers the largest logit (~8.02).
"""

import math
import os
import time
from contextlib import ExitStack

import ml_dtypes
import numpy as np

import concourse.bass as bass
from concourse import bacc
import concourse.mybir as mybir
import concourse.tile as tile
from concourse.bass_utils import run_bass_kernel_spmd

B, S, D = 4, 4096, 2048
HD = 128
DV = 256
DVA = DV + 1      # + ones column for row sums
SK = S // 2       # keys per core
N_CORES = 8
DEPTH = 12
SCALE = HD ** -0.5

DT = D // 128     # 16 d-tiles
SKT = SK // 128   # 16 key tiles per core
KG = SKT // 2     # 8 key groups (DoubleRow pairs)
QC = S // 512     # 8 query chunks
KC = SK // 512    # 4 key chunks (phase-1 k/v projection)

W_SC = 32.0       # W pre-scale before e4m3 (W entries ~N(0, 1/D))
INV_SC = 1.0 / W_SC
ET_BIAS = -3.0    # exp(a - 3): keeps max exp < e4m3 max (224)

F8 = mybir.dt.float8e4
BF16 = mybir.dt.bfloat16
F32 = mybir.dt.float32
DR = mybir.MatmulPerfMode.DoubleRow
MULT = mybir.AluOpType.mult
SUB = mybir.AluOpType.subtract

_cache = {}


def build_nc():
    nc = bacc.Bacc("TRN2", target_bir_lowering=False, debug=False)

    xh_d = nc.declare_dram_parameter("xh", [D, S], F8, isOutput=False)
    xl_d = nc.declare_dram_parameter("xl", [D, S], F8, isOutput=False)
    wh_d = nc.declare_dram_parameter("wh", [D, 3 * DV], F8, isOutput=False)
    wl_d = nc.declare_dram_parameter("wl", [D, 3 * DV], F8, isOutput=False)
    out_d = nc.declare_dram_parameter("out", [2, S, DVA], BF16, isOutput=True)

    # tiled views: [p, dt, col] / [p, row_block, col] so one DMA moves a
    # whole chunk (SP dispatch is 565ns per dma_start — batching matters)
    xh = xh_d.ap().rearrange("(dt p) s -> p dt s", p=128)      # [128, DT, S]
    xl = xl_d.ap().rearrange("(dt p) s -> p dt s", p=128)
    wh = wh_d.ap().rearrange("(dt p) c -> p dt c", p=128)      # [128, DT, 768]
    wl = wl_d.ap().rearrange("(dt p) c -> p dt c", p=128)
    out = out_d.ap().rearrange("h (qs p) d -> h p qs d", p=128)  # [2, 128, 32, DVA]

    with tile.TileContext(nc) as tc, ExitStack() as ctx:
        singles = ctx.enter_context(tc.tile_pool(name="singles", bufs=1))
        x_pool = ctx.enter_context(tc.tile_pool(name="x", bufs=6))
        e_pool = ctx.enter_context(tc.tile_pool(name="e", bufs=8))
        o_pool = ctx.enter_context(tc.tile_pool(name="o", bufs=8))

        w_hi = singles.tile([128, DT, 3 * DV], F8, tag="w_hi")
        w_lo = singles.tile([128, DT, 3 * DV], F8, tag="w_lo")
        kT = singles.tile([128, 2, SK], BF16, tag="kT")       # [dh, head, key]
        qT = singles.tile([128, 2, S], BF16, tag="qT")        # [dh, head, query]
        vhi = singles.tile([128, SKT, DVA], F8, tag="vhi")    # [key_row, kt, dv+1]
        vlo = singles.tile([128, SKT, DVA], F8, tag="vlo")
        vaug = singles.tile([128, SKT, DVA], BF16, tag="vaug")
        bias_t = singles.tile([128, 1], F32, tag="bias")

        nc.vector.memset(bias_t, ET_BIAS)
        nc.vector.memset(vhi[:, :, DV:DVA], 1.0)
        nc.vector.memset(vlo[:, :, DV:DVA], 0.0)
        nc.vector.memset(vaug[:, :, DV:DVA], 1.0)

        nc.sync.dma_start(out=w_hi, in_=wh)
        nc.sync.dma_start(out=w_lo, in_=wl)

        def dma_chunk(sc, name):
            th = x_pool.tile([128, DT, 512], F8, tag="xt", name=f"xh_{name}")
            tl = x_pool.tile([128, DT, 512], F8, tag="xt", name=f"xl_{name}")
            nc.sync.dma_start(out=th, in_=xh[:, :, sc * 512:(sc + 1) * 512])
            nc.sync.dma_start(out=tl, in_=xl[:, :, sc * 512:(sc + 1) * 512])
            return th, tl

        def proj3(ps, th, tl, col0, col1, xslice=None):
            """3-term hi/lo split projection: 24 DoubleRow matmuls into ps.

            If xslice is None: stationary = W cols [col0:col1], moving = x
            (output = [dv_cols, 512 rows]). Else stationary = x col slice,
            moving = W cols (output = [x rows, dv cols])."""
            terms = [(w_hi, th), (w_lo, th), (w_hi, tl)]
            n = len(terms)
            for t, (wt, xt) in enumerate(terms):
                for p in range(DT // 2):
                    first = (t == 0 and p == 0)
                    last = (t == n - 1 and p == DT // 2 - 1)
                    dpair = slice(2 * p, 2 * p + 2)
                    if xslice is None:
                        nc.tensor.matmul(ps, wt[:, dpair, col0:col1], xt[:, dpair, :],
                                         start=first, stop=last, perf_mode=DR)
                    else:
                        nc.tensor.matmul(ps, xt[:, dpair, xslice], wt[:, dpair, col0:col1],
                                         start=first, stop=last, perf_mode=DR)

        # ---- phase 1: k and v projections over this core's key half -----
        pctx = ExitStack()
        pp = pctx.enter_context(tc.tile_pool(name="pp", bufs=6, space=bass.MemorySpace.PSUM))

        # PE p-state warm-up while initial DMAs land
        jt = singles.tile([128, 512], BF16, tag="junk")
        nc.vector.memset(jt, 0.0)
        jps = pp.tile([128, 512], F32, tag="pp", name="jps")
        for _ in range(48):
            nc.tensor.matmul(jps, jt[:, 0:128], jt, start=True, stop=True)

        for sc in range(KC):
            th, tl = dma_chunk(sc, f"p1_{sc}")
            for h in range(2):
                ps = pp.tile([128, 512], F32, tag="pp", name=f"kps{sc}{h}")
                proj3(ps, th, tl, DV + h * HD, DV + (h + 1) * HD)
                nc.vector.tensor_scalar_mul(kT[:, h, sc * 512:(sc + 1) * 512], ps, INV_SC)
            for ssl in range(4):
                idx = sc * 4 + ssl
                vps = pp.tile([128, 512], F32, tag="pp", name=f"vps{idx}")
                proj3(vps[:, 0:DV], th, tl, 2 * DV, 3 * DV,
                      xslice=slice(ssl * 128, (ssl + 1) * 128))
                nc.vector.tensor_scalar_mul(vhi[:, idx, 0:DV], vps[:, 0:DV], INV_SC)
                nc.vector.scalar_tensor_tensor(vlo[:, idx, 0:DV], vps[:, 0:DV], INV_SC,
                                               vhi[:, idx, 0:DV], MULT, SUB)
                nc.vector.tensor_scalar_mul(vaug[:, idx, 0:DV], vps[:, 0:DV], INV_SC)
        pctx.close()

        # ---- phase 2: per query chunk: q proj, scores, exp, PV ----------
        ps_sc = ctx.enter_context(tc.tile_pool(name="ps_sc", bufs=2, space=bass.MemorySpace.PSUM))
        ps_pv = ctx.enter_context(tc.tile_pool(name="ps_pv", bufs=4, space=bass.MemorySpace.PSUM))

        for qc in range(QC):
            th, tl = dma_chunk(qc, f"p2_{qc}")
            qps = ps_sc.tile([128, 2, 512], F32, tag="sc", name=f"qps{qc}")
            for h in range(2):
                proj3(qps[:, h, :], th, tl, h * HD, (h + 1) * HD)
            for h in range(2):
                nc.vector.tensor_scalar_mul(qT[:, h, qc * 512:(qc + 1) * 512], qps[:, h, :], INV_SC)

            qTs = qT[:, 0, qc * 512:(qc + 1) * 512], qT[:, 1, qc * 512:(qc + 1) * 512]
            for h in range(2):
                pvs = [ps_pv.tile([128, DVA], F32, tag="pv", padded_shape=[128, 512],
                                  name=f"pv{qc}{h}{i}") for i in range(4)]
                for g in range(KG):
                    sg = ps_sc.tile([128, 2, 512], F32, tag="sc", name=f"sg{qc}{h}{g}")
                    for i in range(2):
                        kt = 2 * g + i
                        nc.tensor.matmul(sg[:, i, :], kT[:, h, kt * 128:(kt + 1) * 128],
                                         qTs[h], start=True, stop=True)
                    if h == 0:
                        e8 = e_pool.tile([128, 2, 512], F8, tag="e8", name=f"e8_{qc}{g}")
                        nc.scalar.activation(out=e8, in_=sg,
                                             func=mybir.ActivationFunctionType.Exp,
                                             bias=bias_t, scale=SCALE)
                        for qsl in range(4):
                            stat = e8[:, :, qsl * 128:(qsl + 1) * 128]
                            nc.tensor.matmul(pvs[qsl], stat, vhi[:, 2 * g:2 * g + 2, :],
                                             start=(g == 0), stop=False, perf_mode=DR)
                            nc.tensor.matmul(pvs[qsl], stat, vlo[:, 2 * g:2 * g + 2, :],
                                             start=False, stop=(g == KG - 1), perf_mode=DR)
                    else:
                        eb = e_pool.tile([128, 2, 512], BF16, tag="eb", name=f"eb_{qc}{g}")
                        nc.scalar.activation(out=eb, in_=sg,
                                             func=mybir.ActivationFunctionType.Exp,
                                             bias=bias_t, scale=SCALE)
                        for qsl in range(4):
                            for i in range(2):
                                nc.tensor.matmul(pvs[qsl], eb[:, i, qsl * 128:(qsl + 1) * 128],
                                                 vaug[:, 2 * g + i, :],
                                                 start=(g == 0 and i == 0),
                                                 stop=(g == KG - 1 and i == 1))
                onum = o_pool.tile([128, 4, DVA], BF16, tag="onum", name=f"on{qc}{h}")
                for qsl in range(4):
                    nc.vector.tensor_copy(onum[:, qsl, :], pvs[qsl])
                nc.sync.dma_start(out=out[h, :, qc * 4:qc * 4 + 4, :], in_=onum)

    nc.compile()
    return nc


def _lam(lambda_q1, lambda_q2, lambda_k1, lambda_k2):
    lam_init = 0.8 - 0.6 * math.exp(-0.3 * DEPTH)
    l1 = math.exp(float(np.sum(lambda_q1.astype(np.float64) * lambda_k1.astype(np.float64))))
    l2 = math.exp(float(np.sum(lambda_q2.astype(np.float64) * lambda_k2.astype(np.float64))))
    return l1 + l2 + lam_init


def kernel(x, WQ, WK, WV, lambda_q1, lambda_q2, lambda_k1, lambda_k2):
    if "nc" not in _cache:
        _cache["nc"] = build_nc()
    nc = _cache["nc"]

    f8 = ml_dtypes.float8_e4m3
    lam = _lam(lambda_q1, lambda_q2, lambda_k1, lambda_k2)

    w = np.concatenate([np.asarray(WQ, np.float32), np.asarray(WK, np.float32),
                        np.asarray(WV, np.float32)], axis=1) * W_SC   # [D, 768]
    wh = np.ascontiguousarray(w, dtype=f8)
    wl = np.ascontiguousarray(w - wh.astype(np.float32), dtype=f8)

    in_maps = []
    for c in range(N_CORES):
        b, kh = c // 2, c % 2
        xb = np.asarray(x[b], np.float32)
        if kh:
            xb = np.concatenate([xb[SK:], xb[:SK]], axis=0)
        xbh = np.asarray(xb, f8)
        xbl = np.asarray(xb - xbh.astype(np.float32), f8)
        in_maps.append({
            "xh": np.ascontiguousarray(xbh.T),
            "xl": np.ascontiguousarray(xbl.T),
            "wh": wh, "wl": wl,
        })

    kres = None
    for attempt in range(3):
        try:
            kres = run_bass_kernel_spmd(nc, in_maps, list(range(N_CORES)))
            break
        except (ModuleNotFoundError, ImportError):
            os.environ["BASS_NEVER_TRACE"] = "1"
        except Exception:
            if attempt == 2:
                raise
            time.sleep(5)
    if kres is None:
        kres = run_bass_kernel_spmd(nc, in_maps, list(range(N_CORES)))
    _cache["last_results"] = kres
    _cache["input_names"] = ["xh", "xl", "wh", "wl"]
    res = kres.results

    out = np.empty((B, S, DV), np.float32)
    for b in range(B):
        na = res[2 * b]["out"].astype(np.float32)       # [2, S, DVA], canonical q
        nb = res[2 * b + 1]["out"].astype(np.float32)   # q rotated by SK
        nb = np.concatenate([nb[:, SK:, :], nb[:, :SK, :]], axis=1)
        n = na + nb
        o1 = n[0, :, :DV] / n[0, :, DV:DVA]
        o2 = n[1, :, :DV] / n[1, :, DV:DVA]
        out[b] = o1 - lam * o2
    return out


# revision 8
# speedup vs baseline: 1.2774x; 1.1129x over previous
"""Differential attention kernel for 8 Trainium2 NeuronCores — v5.

Key-sharded: core c handles batch c//2 and key-half c%2 (2048 keys), all
4096 queries. Per-head partial numerators + row sums go back to the host,
which adds the two halves and normalizes (softmax denominators span both
cores' keys).

fp8 (e4m3) DoubleRow matmuls carry the projections and head-1 PV:
- projections use a 3-term hi/lo split (x and W each split into e4m3
  hi + residual lo; terms hi*hi + hi*lo + lo*hi) — bf16-level accuracy at
  0.75x the bf16 matmul cost.
- head-1 PV uses single-e4m3 exp scores against hi/lo-split V — half the
  bf16 PV cost. Head 2 (amplified by lambda≈2.8) stays bf16.
- scores stay bf16 (contraction is only 128, DoubleRow needs 256).
exp is biased by -3 so e4m3's max (224) covers the largest logit (~8.02).

Schedule notes: score emission is software-pipelined one key-group ahead
so the in-order PE queue keeps a score matmul in flight while the
activation engine runs exp. The next chunk's q-projections run between
the two head loops, time-sharing the score-psum slots (PSUM is exactly
full: 4 score banks + 4 PV banks). DMAs are whole-chunk single transfers
(the SP engine serializes DMA dispatch at ~0.4-3us each).
"""

import math
import os
import time
from contextlib import ExitStack

import ml_dtypes
import numpy as np

import concourse.bass as bass
from concourse import bacc
import concourse.mybir as mybir
import concourse.tile as tile
from concourse.bass_utils import run_bass_kernel_spmd

B, S, D = 4, 4096, 2048
HD = 128
DV = 256
DVA = DV + 1      # + ones column for row sums
SK = S // 2       # keys per core
N_CORES = 8
DEPTH = 12
SCALE = HD ** -0.5

DT = D // 128     # 16 d-tiles
SKT = SK // 128   # 16 key tiles per core
KG = SKT // 2     # 8 key groups (DoubleRow pairs)
QC = S // 512     # 8 query chunks
KC = SK // 512    # 4 key chunks (phase-1 k/v projection)

W_SC = 32.0       # W pre-scale before e4m3 (W entries ~N(0, 1/D))
INV_SC = 1.0 / W_SC
ET_BIAS = -3.0    # exp(a - 3): keeps max exp < e4m3 max (224)

F8 = mybir.dt.float8e4
BF16 = mybir.dt.bfloat16
F32 = mybir.dt.float32
DR = mybir.MatmulPerfMode.DoubleRow
MULT = mybir.AluOpType.mult
SUB = mybir.AluOpType.subtract

_cache = {}


def build_nc():
    nc = bacc.Bacc("TRN2", target_bir_lowering=False, debug=False)

    xh_d = nc.declare_dram_parameter("xh", [D, S], F8, isOutput=False)
    xl_d = nc.declare_dram_parameter("xl", [D, S], F8, isOutput=False)
    wh_d = nc.declare_dram_parameter("wh", [D, 3 * DV], F8, isOutput=False)
    wl_d = nc.declare_dram_parameter("wl", [D, 3 * DV], F8, isOutput=False)
    out_d = nc.declare_dram_parameter("out", [2, S, DVA], BF16, isOutput=True)

    # tiled views: [p, dt, col] so one DMA moves a whole chunk (the SP
    # engine serializes DMAs; per-dt dma_starts would swamp it)
    xh = xh_d.ap().rearrange("(dt p) s -> p dt s", p=128)      # [128, DT, S]
    xl = xl_d.ap().rearrange("(dt p) s -> p dt s", p=128)
    wh = wh_d.ap().rearrange("(dt p) c -> p dt c", p=128)      # [128, DT, 768]
    wl = wl_d.ap().rearrange("(dt p) c -> p dt c", p=128)
    out = out_d.ap().rearrange("h (qs p) d -> h p qs d", p=128)  # [2, 128, 32, DVA]

    with tile.TileContext(nc) as tc, ExitStack() as ctx:
        singles = ctx.enter_context(tc.tile_pool(name="singles", bufs=1))
        x_pool = ctx.enter_context(tc.tile_pool(name="x", bufs=10))
        e_pool = ctx.enter_context(tc.tile_pool(name="e", bufs=4))
        o_pool = ctx.enter_context(tc.tile_pool(name="o", bufs=4))

        w_hi = singles.tile([128, DT, 3 * DV], F8, tag="w_hi")
        w_lo = singles.tile([128, DT, 3 * DV], F8, tag="w_lo")
        kT = singles.tile([128, 2, SK], BF16, tag="kT")       # [dh, head, key]
        qT = singles.tile([128, 2, S], BF16, tag="qT")        # [dh, head, query]
        vhi = singles.tile([128, SKT, DVA], F8, tag="vhi")    # [key_row, kt, dv+1]
        vlo = singles.tile([128, SKT, DVA], F8, tag="vlo")
        vaug = singles.tile([128, SKT, DVA], BF16, tag="vaug")
        bias_t = singles.tile([128, 1], F32, tag="bias")

        nc.vector.memset(bias_t, ET_BIAS)
        nc.vector.memset(vhi[:, :, DV:DVA], 1.0)
        nc.vector.memset(vlo[:, :, DV:DVA], 0.0)
        nc.vector.memset(vaug[:, :, DV:DVA], 1.0)

        chunks = {}

        def dma_chunk(sc):
            th = x_pool.tile([128, DT, 512], F8, tag="xt", name=f"xh_{sc}")
            tl = x_pool.tile([128, DT, 512], F8, tag="xt", name=f"xl_{sc}")
            nc.sync.dma_start(out=th, in_=xh[:, :, sc * 512:(sc + 1) * 512])
            nc.sync.dma_start(out=tl, in_=xl[:, :, sc * 512:(sc + 1) * 512])
            chunks[sc] = (th, tl)

        # K weights first, then chunk 0, V weights, chunk 1, Q weights last
        # (q-projection happens in phase 2) — SP runs DMAs strictly in order.
        for c0, c1 in ((DV, 2 * DV),):
            nc.sync.dma_start(out=w_hi[:, :, c0:c1], in_=wh[:, :, c0:c1])
            nc.sync.dma_start(out=w_lo[:, :, c0:c1], in_=wl[:, :, c0:c1])
        dma_chunk(0)
        for c0, c1 in ((2 * DV, 3 * DV),):
            nc.sync.dma_start(out=w_hi[:, :, c0:c1], in_=wh[:, :, c0:c1])
            nc.sync.dma_start(out=w_lo[:, :, c0:c1], in_=wl[:, :, c0:c1])
        dma_chunk(1)

        def proj3(ps, sc, col0, col1, xslice=None):
            """3-term hi/lo split projection: 24 DoubleRow matmuls into ps."""
            th, tl = chunks[sc]
            terms = [(w_hi, th), (w_lo, th), (w_hi, tl)]
            for t, (wt, xt) in enumerate(terms):
                for p in range(DT // 2):
                    first = (t == 0 and p == 0)
                    last = (t == len(terms) - 1 and p == DT // 2 - 1)
                    dpair = slice(2 * p, 2 * p + 2)
                    if xslice is None:
                        nc.tensor.matmul(ps, wt[:, dpair, col0:col1], xt[:, dpair, :],
                                         start=first, stop=last, perf_mode=DR)
                    else:
                        nc.tensor.matmul(ps, xt[:, dpair, xslice], wt[:, dpair, col0:col1],
                                         start=first, stop=last, perf_mode=DR)

        # ---- phase 1: k and v projections over this core's key half -----
        pctx = ExitStack()
        pp = pctx.enter_context(tc.tile_pool(name="pp", bufs=6, space=bass.MemorySpace.PSUM))

        # PE p-state warm-up while initial DMAs land
        jt = singles.tile([128, 512], BF16, tag="junk")
        nc.vector.memset(jt, 0.0)
        jps = pp.tile([128, 512], F32, tag="pp", name="jps")
        for _ in range(14):
            nc.tensor.matmul(jps, jt[:, 0:128], jt, start=True, stop=True)

        for sc in range(KC):
            if sc + 2 < KC:
                dma_chunk(sc + 2)
            elif sc == KC - 2:
                for c0, c1 in ((0, DV),):
                    nc.sync.dma_start(out=w_hi[:, :, c0:c1], in_=wh[:, :, c0:c1])
                    nc.sync.dma_start(out=w_lo[:, :, c0:c1], in_=wl[:, :, c0:c1])
            for h in range(2):
                ps = pp.tile([128, 512], F32, tag="pp", name=f"kps{sc}{h}")
                proj3(ps, sc, DV + h * HD, DV + (h + 1) * HD)
                nc.vector.tensor_scalar_mul(kT[:, h, sc * 512:(sc + 1) * 512], ps, INV_SC)
            for ssl in range(4):
                idx = sc * 4 + ssl
                vps = pp.tile([128, 512], F32, tag="pp", name=f"vps{idx}")
                proj3(vps[:, 0:DV], sc, 2 * DV, 3 * DV,
                      xslice=slice(ssl * 128, (ssl + 1) * 128))
                nc.vector.tensor_scalar_mul(vhi[:, idx, 0:DV], vps[:, 0:DV], INV_SC)
                nc.vector.scalar_tensor_tensor(vlo[:, idx, 0:DV], vps[:, 0:DV], INV_SC,
                                               vhi[:, idx, 0:DV], MULT, SUB)
                nc.vector.tensor_scalar_mul(vaug[:, idx, 0:DV], vps[:, 0:DV], INV_SC)
        pctx.close()

        # ---- phase 2: per query chunk: scores, exp, PV ------------------
        ps_sc = ctx.enter_context(tc.tile_pool(name="ps_sc", bufs=2, space=bass.MemorySpace.PSUM))
        ps_pv = ctx.enter_context(tc.tile_pool(name="ps_pv", bufs=4, space=bass.MemorySpace.PSUM))

        def qproj_block(qcn, h):
            # time-shares a score-psum slot (PSUM has no spare bank)
            qp = ps_sc.tile([128, 2, 512], F32, tag="sc", name=f"qp{qcn}{h}")[:, 0, :]
            proj3(qp, qcn, h * HD, (h + 1) * HD)
            nc.vector.tensor_scalar_mul(qT[:, h, qcn * 512:(qcn + 1) * 512], qp, INV_SC)

        qproj_block(0, 0)
        qproj_block(0, 1)

        for qc in range(QC):
            if qc + 1 < QC and (qc + 1) not in chunks:
                dma_chunk(qc + 1)
            for h in range(2):
                pvs = [ps_pv.tile([128, DVA], F32, tag="pv", name=f"pv{qc}{h}{i}")
                       for i in range(4)]
                qTs = qT[:, h, qc * 512:(qc + 1) * 512]

                def scores(g, h=h, qTs=qTs, qc=qc):
                    sg = ps_sc.tile([128, 2, 512], F32, tag="sc", name=f"sg{qc}{h}{g}")
                    for i in range(2):
                        kt = 2 * g + i
                        nc.tensor.matmul(sg[:, i, :], kT[:, h, kt * 128:(kt + 1) * 128],
                                         qTs, start=True, stop=True)
                    return sg

                sg_prev = scores(0)
                for g in range(KG):
                    sg_next = scores(g + 1) if g + 1 < KG else None
                    first, last = (g == 0), (g == KG - 1)
                    if h == 0:
                        e8 = e_pool.tile([128, 2, 512], F8, tag="e8", name=f"e8_{qc}{g}")
                        nc.scalar.activation(out=e8, in_=sg_prev,
                                             func=mybir.ActivationFunctionType.Exp,
                                             bias=bias_t, scale=SCALE)
                        for qsl in range(4):
                            stat = e8[:, :, qsl * 128:(qsl + 1) * 128]
                            nc.tensor.matmul(pvs[qsl], stat, vhi[:, 2 * g:2 * g + 2, :],
                                             start=first, stop=False, perf_mode=DR)
                            nc.tensor.matmul(pvs[qsl], stat, vlo[:, 2 * g:2 * g + 2, :],
                                             start=False, stop=last, perf_mode=DR)
                    else:
                        eb = e_pool.tile([128, 2, 512], BF16, tag="eb", name=f"eb_{qc}{g}")
                        nc.scalar.activation(out=eb, in_=sg_prev,
                                             func=mybir.ActivationFunctionType.Exp,
                                             bias=bias_t, scale=SCALE)
                        for qsl in range(4):
                            for i in range(2):
                                st = eb[:, i, qsl * 128:(qsl + 1) * 128]
                                nc.tensor.matmul(pvs[qsl], st, vaug[:, 2 * g + i, :],
                                                 start=(first and i == 0),
                                                 stop=(last and i == 1))
                    sg_prev = sg_next

                onum = o_pool.tile([128, 4, DVA], BF16, tag="onum", name=f"on{qc}{h}")
                for qsl in range(4):
                    nc.vector.tensor_copy(onum[:, qsl, :], pvs[qsl])
                nc.sync.dma_start(out=out[h, :, qc * 4:qc * 4 + 4, :], in_=onum)

                if h == 0 and qc + 1 < QC:
                    qproj_block(qc + 1, 0)
                    qproj_block(qc + 1, 1)

    nc.compile()
    return nc


def _lam(lambda_q1, lambda_q2, lambda_k1, lambda_k2):
    lam_init = 0.8 - 0.6 * math.exp(-0.3 * DEPTH)
    l1 = math.exp(float(np.sum(lambda_q1.astype(np.float64) * lambda_k1.astype(np.float64))))
    l2 = math.exp(float(np.sum(lambda_q2.astype(np.float64) * lambda_k2.astype(np.float64))))
    return l1 + l2 + lam_init


def kernel(x, WQ, WK, WV, lambda_q1, lambda_q2, lambda_k1, lambda_k2):
    if "nc" not in _cache:
        _cache["nc"] = build_nc()
    nc = _cache["nc"]

    f8 = ml_dtypes.float8_e4m3
    lam = _lam(lambda_q1, lambda_q2, lambda_k1, lambda_k2)

    w = np.concatenate([np.asarray(WQ, np.float32), np.asarray(WK, np.float32),
                        np.asarray(WV, np.float32)], axis=1) * W_SC   # [D, 768]
    wh = np.ascontiguousarray(w, dtype=f8)
    wl = np.ascontiguousarray(w - wh.astype(np.float32), dtype=f8)

    in_maps = []
    for c in range(N_CORES):
        b, kh = c // 2, c % 2
        xb = np.asarray(x[b], np.float32)
        if kh:
            xb = np.concatenate([xb[SK:], xb[:SK]], axis=0)
        xbh = np.asarray(xb, f8)
        xbl = np.asarray(xb - xbh.astype(np.float32), f8)
        in_maps.append({
            "xh": np.ascontiguousarray(xbh.T),
            "xl": np.ascontiguousarray(xbl.T),
            "wh": wh, "wl": wl,
        })

    kres = None
    for attempt in range(3):
        try:
            kres = run_bass_kernel_spmd(nc, in_maps, list(range(N_CORES)))
            break
        except (ModuleNotFoundError, ImportError):
            os.environ["BASS_NEVER_TRACE"] = "1"
        except Exception:
            if attempt == 2:
                raise
            time.sleep(5)
    if kres is None:
        kres = run_bass_kernel_spmd(nc, in_maps, list(range(N_CORES)))
    _cache["last_results"] = kres
    _cache["input_names"] = ["xh", "xl", "wh", "wl"]
    res = kres.results

    out = np.empty((B, S, DV), np.float32)
    for b in range(B):
        na = res[2 * b]["out"].astype(np.float32)       # [2, S, DVA], canonical q
        nb = res[2 * b + 1]["out"].astype(np.float32)   # q rotated by SK
        nb = np.concatenate([nb[:, SK:, :], nb[:, :SK, :]], axis=1)
        n = na + nb
        o1 = n[0, :, :DV] / n[0, :, DV:DVA]
        o2 = n[1, :, :DV] / n[1, :, DV:DVA]
        out[b] = o1 - lam * o2
    return out


# revision 9
# speedup vs baseline: 1.3083x; 1.0242x over previous
"""Differential attention kernel for 8 Trainium2 NeuronCores — v5.

Key-sharded: core c handles batch c//2 and key-half c%2 (2048 keys), all
4096 queries. Per-head partial numerators + row sums go back to the host,
which adds the two halves and normalizes (softmax denominators span both
cores' keys).

fp8 (e4m3) DoubleRow matmuls carry the projections and head-1 PV:
- projections use a 3-term hi/lo split (x and W each split into e4m3
  hi + residual lo; terms hi*hi + hi*lo + lo*hi) — bf16-level accuracy at
  0.75x the bf16 matmul cost.
- head-1 PV uses single-e4m3 exp scores against hi/lo-split V — half the
  bf16 PV cost. Head 2 (amplified by lambda≈2.8) stays bf16.
- scores stay bf16 (contraction is only 128, DoubleRow needs 256).
exp is biased by -3 so e4m3's max (224) covers the largest logit (~8.02).

Schedule notes: score emission is software-pipelined one key-group ahead
so the in-order PE queue keeps a score matmul in flight while the
activation engine runs exp. The next chunk's q-projections run between
the two head loops, time-sharing the score-psum slots (PSUM is exactly
full: 4 score banks + 4 PV banks). DMAs are whole-chunk single transfers
(the SP engine serializes DMA dispatch at ~0.4-3us each).
"""

import math
import os
import time
from contextlib import ExitStack

import ml_dtypes
import numpy as np

import concourse.bass as bass
from concourse import bacc
import concourse.mybir as mybir
import concourse.tile as tile
from concourse.bass_utils import run_bass_kernel_spmd

B, S, D = 4, 4096, 2048
HD = 128
DV = 256
DVA = DV + 1      # + ones column for row sums
SK = S // 2       # keys per core
N_CORES = 8
DEPTH = 12
SCALE = HD ** -0.5

DT = D // 128     # 16 d-tiles
SKT = SK // 128   # 16 key tiles per core
KG = SKT // 2     # 8 key groups (DoubleRow pairs)
QC = S // 512     # 8 query chunks
KC = SK // 512    # 4 key chunks (phase-1 k/v projection)

W_SC = 32.0       # W pre-scale before e4m3 (W entries ~N(0, 1/D))
INV_SC = 1.0 / W_SC
ET_BIAS = -3.0    # exp(a - 3): keeps max exp < e4m3 max (224)

F8 = mybir.dt.float8e4
BF16 = mybir.dt.bfloat16
F32 = mybir.dt.float32
DR = mybir.MatmulPerfMode.DoubleRow
MULT = mybir.AluOpType.mult
SUB = mybir.AluOpType.subtract

_cache = {}


def build_nc():
    nc = bacc.Bacc("TRN2", target_bir_lowering=False, debug=False)

    xh_d = nc.declare_dram_parameter("xh", [D, S], F8, isOutput=False)
    xl_d = nc.declare_dram_parameter("xl", [D, S], F8, isOutput=False)
    wh_d = nc.declare_dram_parameter("wh", [D, 3 * DV], F8, isOutput=False)
    wl_d = nc.declare_dram_parameter("wl", [D, 3 * DV], F8, isOutput=False)
    out_d = nc.declare_dram_parameter("out", [2, S, DVA], BF16, isOutput=True)

    # tiled views: [p, dt, col] so one DMA moves a whole chunk (the SP
    # engine serializes DMAs; per-dt dma_starts would swamp it)
    xh = xh_d.ap().rearrange("(dt p) s -> p dt s", p=128)      # [128, DT, S]
    xl = xl_d.ap().rearrange("(dt p) s -> p dt s", p=128)
    wh = wh_d.ap().rearrange("(dt p) c -> p dt c", p=128)      # [128, DT, 768]
    wl = wl_d.ap().rearrange("(dt p) c -> p dt c", p=128)
    out = out_d.ap().rearrange("h (qs p) d -> h p qs d", p=128)  # [2, 128, 32, DVA]

    with tile.TileContext(nc) as tc, ExitStack() as ctx:
        singles = ctx.enter_context(tc.tile_pool(name="singles", bufs=1))
        x_pool = ctx.enter_context(tc.tile_pool(name="x", bufs=10))
        e_pool = ctx.enter_context(tc.tile_pool(name="e", bufs=4))
        o_pool = ctx.enter_context(tc.tile_pool(name="o", bufs=4))

        w_hi = singles.tile([128, DT, 3 * DV], F8, tag="w_hi")
        w_lo = singles.tile([128, DT, 3 * DV], F8, tag="w_lo")
        kT = singles.tile([128, 2, SK], BF16, tag="kT")       # [dh, head, key]
        qT = singles.tile([128, 2, S], BF16, tag="qT")        # [dh, head, query]
        vhi = singles.tile([128, SKT, DVA], F8, tag="vhi")    # [key_row, kt, dv+1]
        vlo = singles.tile([128, SKT, DVA], F8, tag="vlo")
        vaug = singles.tile([128, SKT, DVA], BF16, tag="vaug")
        bias_t = singles.tile([128, 1], F32, tag="bias")

        nc.vector.memset(bias_t, ET_BIAS)
        nc.vector.memset(vhi[:, :, DV:DVA], 1.0)
        nc.vector.memset(vlo[:, :, DV:DVA], 0.0)
        nc.vector.memset(vaug[:, :, DV:DVA], 1.0)

        chunks = {}

        def dma_chunk(sc):
            th = x_pool.tile([128, DT, 512], F8, tag="xt", name=f"xh_{sc}")
            tl = x_pool.tile([128, DT, 512], F8, tag="xt", name=f"xl_{sc}")
            nc.gpsimd.dma_start(out=th, in_=xh[:, :, sc * 512:(sc + 1) * 512])
            nc.gpsimd.dma_start(out=tl, in_=xl[:, :, sc * 512:(sc + 1) * 512])
            chunks[sc] = (th, tl)

        # K weights first, then chunk 0, V weights, chunk 1, Q weights last
        # (q-projection happens in phase 2) — SP runs DMAs strictly in order.
        for c0, c1 in ((DV, 2 * DV),):
            nc.sync.dma_start(out=w_hi[:, :, c0:c1], in_=wh[:, :, c0:c1])
            nc.sync.dma_start(out=w_lo[:, :, c0:c1], in_=wl[:, :, c0:c1])
        dma_chunk(0)
        for c0, c1 in ((2 * DV, 3 * DV),):
            nc.sync.dma_start(out=w_hi[:, :, c0:c1], in_=wh[:, :, c0:c1])
            nc.sync.dma_start(out=w_lo[:, :, c0:c1], in_=wl[:, :, c0:c1])
        dma_chunk(1)

        def proj3(ps, sc, col0, col1, xslice=None):
            """3-term hi/lo split projection: 24 DoubleRow matmuls into ps."""
            th, tl = chunks[sc]
            terms = [(w_hi, th), (w_lo, th), (w_hi, tl)]
            for t, (wt, xt) in enumerate(terms):
                for p in range(DT // 2):
                    first = (t == 0 and p == 0)
                    last = (t == len(terms) - 1 and p == DT // 2 - 1)
                    dpair = slice(2 * p, 2 * p + 2)
                    if xslice is None:
                        nc.tensor.matmul(ps, wt[:, dpair, col0:col1], xt[:, dpair, :],
                                         start=first, stop=last, perf_mode=DR)
                    else:
                        nc.tensor.matmul(ps, xt[:, dpair, xslice], wt[:, dpair, col0:col1],
                                         start=first, stop=last, perf_mode=DR)

        # ---- phase 1: k and v projections over this core's key half -----
        pctx = ExitStack()
        pp = pctx.enter_context(tc.tile_pool(name="pp", bufs=6, space=bass.MemorySpace.PSUM))

        # PE p-state warm-up while initial DMAs land
        jt = singles.tile([128, 512], BF16, tag="junk")
        nc.vector.memset(jt, 0.0)
        jps = pp.tile([128, 512], F32, tag="pp", name="jps")
        for _ in range(14):
            nc.tensor.matmul(jps, jt[:, 0:128], jt, start=True, stop=True)

        for sc in range(KC):
            if sc + 2 < KC:
                dma_chunk(sc + 2)
            elif sc == KC - 2:
                for c0, c1 in ((0, DV),):
                    nc.sync.dma_start(out=w_hi[:, :, c0:c1], in_=wh[:, :, c0:c1])
                    nc.sync.dma_start(out=w_lo[:, :, c0:c1], in_=wl[:, :, c0:c1])
            for h in range(2):
                ps = pp.tile([128, 512], F32, tag="pp", name=f"kps{sc}{h}")
                proj3(ps, sc, DV + h * HD, DV + (h + 1) * HD)
                nc.vector.tensor_scalar_mul(kT[:, h, sc * 512:(sc + 1) * 512], ps, INV_SC)
            for ssl in range(4):
                idx = sc * 4 + ssl
                vps = pp.tile([128, 512], F32, tag="pp", name=f"vps{idx}")
                proj3(vps[:, 0:DV], sc, 2 * DV, 3 * DV,
                      xslice=slice(ssl * 128, (ssl + 1) * 128))
                nc.vector.tensor_scalar_mul(vhi[:, idx, 0:DV], vps[:, 0:DV], INV_SC)
                nc.vector.scalar_tensor_tensor(vlo[:, idx, 0:DV], vps[:, 0:DV], INV_SC,
                                               vhi[:, idx, 0:DV], MULT, SUB)
                nc.vector.tensor_scalar_mul(vaug[:, idx, 0:DV], vps[:, 0:DV], INV_SC)
        pctx.close()

        # ---- phase 2: per query chunk: scores, exp, PV ------------------
        ps_sc = ctx.enter_context(tc.tile_pool(name="ps_sc", bufs=2, space=bass.MemorySpace.PSUM))
        ps_pv = ctx.enter_context(tc.tile_pool(name="ps_pv", bufs=4, space=bass.MemorySpace.PSUM))

        def qproj_block(qcn, h):
            # time-shares a score-psum slot (PSUM has no spare bank)
            qp = ps_sc.tile([128, 2, 512], F32, tag="sc", name=f"qp{qcn}{h}")[:, 0, :]
            proj3(qp, qcn, h * HD, (h + 1) * HD)
            nc.vector.tensor_scalar_mul(qT[:, h, qcn * 512:(qcn + 1) * 512], qp, INV_SC)

        qproj_block(0, 0)
        qproj_block(0, 1)

        for qc in range(QC):
            if qc + 1 < QC and (qc + 1) not in chunks:
                dma_chunk(qc + 1)
            for h in range(2):
                pv_t = ps_pv.tile([128, 4, 512], F32, tag="pv", bufs=1, name=f"pv{qc}{h}")
                pvs = [pv_t[:, i, 0:DVA] for i in range(4)]
                qTs = qT[:, h, qc * 512:(qc + 1) * 512]

                def scores(g, h=h, qTs=qTs, qc=qc):
                    sg = ps_sc.tile([128, 2, 512], F32, tag="sc", name=f"sg{qc}{h}{g}")
                    for i in range(2):
                        kt = 2 * g + i
                        nc.tensor.matmul(sg[:, i, :], kT[:, h, kt * 128:(kt + 1) * 128],
                                         qTs, start=True, stop=True)
                    return sg

                sg_prev = scores(0)
                for g in range(KG):
                    sg_next = scores(g + 1) if g + 1 < KG else None
                    first, last = (g == 0), (g == KG - 1)
                    if h == 0:
                        e8 = e_pool.tile([128, 2, 512], F8, tag="e8", name=f"e8_{qc}{g}")
                        nc.scalar.activation(out=e8, in_=sg_prev,
                                             func=mybir.ActivationFunctionType.Exp,
                                             bias=bias_t, scale=SCALE)
                        for qsl in range(4):
                            stat = e8[:, :, qsl * 128:(qsl + 1) * 128]
                            nc.tensor.matmul(pvs[qsl], stat, vhi[:, 2 * g:2 * g + 2, :],
                                             start=first, stop=False, perf_mode=DR)
                            nc.tensor.matmul(pvs[qsl], stat, vlo[:, 2 * g:2 * g + 2, :],
                                             start=False, stop=last, perf_mode=DR)
                    else:
                        eb = e_pool.tile([128, 2, 512], BF16, tag="eb", name=f"eb_{qc}{g}")
                        nc.scalar.activation(out=eb, in_=sg_prev,
                                             func=mybir.ActivationFunctionType.Exp,
                                             bias=bias_t, scale=SCALE)
                        for qsl in range(4):
                            for i in range(2):
                                st = eb[:, i, qsl * 128:(qsl + 1) * 128]
                                nc.tensor.matmul(pvs[qsl], st, vaug[:, 2 * g + i, :],
                                                 start=(first and i == 0),
                                                 stop=(last and i == 1))
                    sg_prev = sg_next

                onum = o_pool.tile([128, 4, DVA], BF16, tag="onum", name=f"on{qc}{h}")
                nc.vector.tensor_copy(onum, pv_t[:, :, 0:DVA])
                nc.sync.dma_start(out=out[h, :, qc * 4:qc * 4 + 4, :], in_=onum)

                if h == 0 and qc + 1 < QC:
                    qproj_block(qc + 1, 0)
                    qproj_block(qc + 1, 1)

    nc.compile()
    return nc


def _lam(lambda_q1, lambda_q2, lambda_k1, lambda_k2):
    lam_init = 0.8 - 0.6 * math.exp(-0.3 * DEPTH)
    l1 = math.exp(float(np.sum(lambda_q1.astype(np.float64) * lambda_k1.astype(np.float64))))
    l2 = math.exp(float(np.sum(lambda_q2.astype(np.float64) * lambda_k2.astype(np.float64))))
    return l1 + l2 + lam_init


def kernel(x, WQ, WK, WV, lambda_q1, lambda_q2, lambda_k1, lambda_k2):
    if "nc" not in _cache:
        _cache["nc"] = build_nc()
    nc = _cache["nc"]

    f8 = ml_dtypes.float8_e4m3
    lam = _lam(lambda_q1, lambda_q2, lambda_k1, lambda_k2)

    w = np.concatenate([np.asarray(WQ, np.float32), np.asarray(WK, np.float32),
                        np.asarray(WV, np.float32)], axis=1) * W_SC   # [D, 768]
    wh = np.ascontiguousarray(w, dtype=f8)
    wl = np.ascontiguousarray(w - wh.astype(np.float32), dtype=f8)

    in_maps = []
    for c in range(N_CORES):
        b, kh = c // 2, c % 2
        xb = np.asarray(x[b], np.float32)
        if kh:
            xb = np.concatenate([xb[SK:], xb[:SK]], axis=0)
        xbh = np.asarray(xb, f8)
        xbl = np.asarray(xb - xbh.astype(np.float32), f8)
        in_maps.append({
            "xh": np.ascontiguousarray(xbh.T),
            "xl": np.ascontiguousarray(xbl.T),
            "wh": wh, "wl": wl,
        })

    kres = None
    for attempt in range(3):
        try:
            kres = run_bass_kernel_spmd(nc, in_maps, list(range(N_CORES)))
            break
        except (ModuleNotFoundError, ImportError):
            os.environ["BASS_NEVER_TRACE"] = "1"
        except Exception:
            if attempt == 2:
                raise
            time.sleep(5)
    if kres is None:
        kres = run_bass_kernel_spmd(nc, in_maps, list(range(N_CORES)))
    _cache["last_results"] = kres
    _cache["input_names"] = ["xh", "xl", "wh", "wl"]
    res = kres.results

    out = np.empty((B, S, DV), np.float32)
    for b in range(B):
        na = res[2 * b]["out"].astype(np.float32)       # [2, S, DVA], canonical q
        nb = res[2 * b + 1]["out"].astype(np.float32)   # q rotated by SK
        nb = np.concatenate([nb[:, SK:, :], nb[:, :SK, :]], axis=1)
        n = na + nb
        o1 = n[0, :, :DV] / n[0, :, DV:DVA]
        o2 = n[1, :, :DV] / n[1, :, DV:DVA]
        out[b] = o1 - lam * o2
    return out


# revision 10
# speedup vs baseline: 1.3208x; 1.0096x over previous
"""Differential attention kernel for 8 Trainium2 NeuronCores — v5.

Key-sharded: core c handles batch c//2 and key-half c%2 (2048 keys), all
4096 queries. Per-head partial numerators + row sums go back to the host,
which adds the two halves and normalizes (softmax denominators span both
cores' keys).

fp8 (e4m3) DoubleRow matmuls carry the projections and head-1 PV:
- projections use a 3-term hi/lo split (x and W each split into e4m3
  hi + residual lo; terms hi*hi + hi*lo + lo*hi) — bf16-level accuracy at
  0.75x the bf16 matmul cost.
- head-1 PV uses single-e4m3 exp scores against hi/lo-split V — half the
  bf16 PV cost. Head 2 (amplified by lambda≈2.8) stays bf16.
- scores stay bf16 (contraction is only 128, DoubleRow needs 256).
exp is biased by -3 so e4m3's max (224) covers the largest logit (~8.02).

Schedule notes: score emission is software-pipelined one key-group ahead
so the in-order PE queue keeps a score matmul in flight while the
activation engine runs exp. The next chunk's q-projections run between
the two head loops, time-sharing the score-psum slots (PSUM is exactly
full: 4 score banks + 4 PV banks). DMAs are whole-chunk single transfers
(the SP engine serializes DMA dispatch at ~0.4-3us each).
"""

import math
import os
import time
from contextlib import ExitStack

import ml_dtypes
import numpy as np

import concourse.bass as bass
from concourse import bacc
import concourse.mybir as mybir
import concourse.tile as tile
from concourse.bass_utils import run_bass_kernel_spmd

B, S, D = 4, 4096, 2048
HD = 128
DV = 256
DVA = DV + 1      # + ones column for row sums
SK = S // 2       # keys per core
N_CORES = 8
DEPTH = 12
SCALE = HD ** -0.5

DT = D // 128     # 16 d-tiles
SKT = SK // 128   # 16 key tiles per core
KG = SKT // 2     # 8 key groups (DoubleRow pairs)
QC = S // 512     # 8 query chunks
KC = SK // 512    # 4 key chunks (phase-1 k/v projection)

W_SC = 32.0       # W pre-scale before e4m3 (W entries ~N(0, 1/D))
INV_SC = 1.0 / W_SC
ET_BIAS = -3.0    # exp(a - 3): keeps max exp < e4m3 max (224)

F8 = mybir.dt.float8e4
BF16 = mybir.dt.bfloat16
F32 = mybir.dt.float32
DR = mybir.MatmulPerfMode.DoubleRow
MULT = mybir.AluOpType.mult
SUB = mybir.AluOpType.subtract

_cache = {}


def build_nc():
    nc = bacc.Bacc("TRN2", target_bir_lowering=False, debug=False)

    xh_d = nc.declare_dram_parameter("xh", [D, S], F8, isOutput=False)
    xl_d = nc.declare_dram_parameter("xl", [D, S], F8, isOutput=False)
    wh_d = nc.declare_dram_parameter("wh", [D, 3 * DV], F8, isOutput=False)
    wl_d = nc.declare_dram_parameter("wl", [D, 3 * DV], F8, isOutput=False)
    out_d = nc.declare_dram_parameter("out", [2, S, DVA], BF16, isOutput=True)

    # tiled views: [p, dt, col] so one DMA moves a whole chunk (the SP
    # engine serializes DMAs; per-dt dma_starts would swamp it)
    xh = xh_d.ap().rearrange("(dt p) s -> p dt s", p=128)      # [128, DT, S]
    xl = xl_d.ap().rearrange("(dt p) s -> p dt s", p=128)
    wh = wh_d.ap().rearrange("(dt p) c -> p dt c", p=128)      # [128, DT, 768]
    wl = wl_d.ap().rearrange("(dt p) c -> p dt c", p=128)
    out = out_d.ap().rearrange("h (qs p) d -> h p qs d", p=128)  # [2, 128, 32, DVA]

    with tile.TileContext(nc) as tc, ExitStack() as ctx:
        singles = ctx.enter_context(tc.tile_pool(name="singles", bufs=1))
        x_pool = ctx.enter_context(tc.tile_pool(name="x", bufs=10))
        e_pool = ctx.enter_context(tc.tile_pool(name="e", bufs=4))
        o_pool = ctx.enter_context(tc.tile_pool(name="o", bufs=4))

        w_hi = singles.tile([128, DT, 3 * DV], F8, tag="w_hi")
        w_lo = singles.tile([128, DT, 3 * DV], F8, tag="w_lo")
        kT = singles.tile([128, 2, SK], BF16, tag="kT")       # [dh, head, key]
        qT = singles.tile([128, 2, S], BF16, tag="qT")        # [dh, head, query]
        vhi = singles.tile([128, SKT, DVA], F8, tag="vhi")    # [key_row, kt, dv+1]
        vlo = singles.tile([128, SKT, DVA], F8, tag="vlo")
        vaug = singles.tile([128, SKT, DVA], BF16, tag="vaug")
        bias_t = singles.tile([128, 1], F32, tag="bias")

        nc.vector.memset(bias_t, ET_BIAS)
        nc.vector.memset(vhi[:, :, DV:DVA], 1.0)
        nc.vector.memset(vlo[:, :, DV:DVA], 0.0)
        nc.vector.memset(vaug[:, :, DV:DVA], 1.0)

        chunks = {}

        def dma_chunk(sc):
            th = x_pool.tile([128, DT, 512], F8, tag="xt", name=f"xh_{sc}")
            tl = x_pool.tile([128, DT, 512], F8, tag="xt", name=f"xl_{sc}")
            nc.gpsimd.dma_start(out=th, in_=xh[:, :, sc * 512:(sc + 1) * 512])
            nc.gpsimd.dma_start(out=tl, in_=xl[:, :, sc * 512:(sc + 1) * 512])
            chunks[sc] = (th, tl)

        # K weights first, then chunk 0, V weights, chunk 1, Q weights last
        # (q-projection happens in phase 2) — SP runs DMAs strictly in order.
        for c0, c1 in ((DV, 2 * DV),):
            nc.sync.dma_start(out=w_hi[:, :, c0:c1], in_=wh[:, :, c0:c1])
            nc.sync.dma_start(out=w_lo[:, :, c0:c1], in_=wl[:, :, c0:c1])
        dma_chunk(0)
        for c0, c1 in ((2 * DV, 3 * DV),):
            nc.sync.dma_start(out=w_hi[:, :, c0:c1], in_=wh[:, :, c0:c1])
            nc.sync.dma_start(out=w_lo[:, :, c0:c1], in_=wl[:, :, c0:c1])
        dma_chunk(1)

        def proj3(ps, sc, col0, col1, xslice=None):
            """3-term hi/lo split projection: 24 DoubleRow matmuls into ps."""
            th, tl = chunks[sc]
            terms = [(w_hi, th), (w_lo, th), (w_hi, tl)]
            for t, (wt, xt) in enumerate(terms):
                for p in range(DT // 2):
                    first = (t == 0 and p == 0)
                    last = (t == len(terms) - 1 and p == DT // 2 - 1)
                    dpair = slice(2 * p, 2 * p + 2)
                    if xslice is None:
                        nc.tensor.matmul(ps, wt[:, dpair, col0:col1], xt[:, dpair, :],
                                         start=first, stop=last, perf_mode=DR)
                    else:
                        nc.tensor.matmul(ps, xt[:, dpair, xslice], wt[:, dpair, col0:col1],
                                         start=first, stop=last, perf_mode=DR)

        # ---- phase 1: k and v projections over this core's key half -----
        pctx = ExitStack()
        pp = pctx.enter_context(tc.tile_pool(name="pp", bufs=6, space=bass.MemorySpace.PSUM))

        # PE p-state warm-up while initial DMAs land
        jt = singles.tile([128, 512], BF16, tag="junk")
        nc.vector.memset(jt, 0.0)
        jps = pp.tile([128, 512], F32, tag="pp", name="jps")
        for _ in range(14):
            nc.tensor.matmul(jps, jt[:, 0:128], jt, start=True, stop=True)

        for sc in range(KC):
            if sc + 2 < KC:
                dma_chunk(sc + 2)
            elif sc == KC - 2:
                for c0, c1 in ((0, DV),):
                    nc.sync.dma_start(out=w_hi[:, :, c0:c1], in_=wh[:, :, c0:c1])
                    nc.sync.dma_start(out=w_lo[:, :, c0:c1], in_=wl[:, :, c0:c1])
            for h in range(2):
                ps = pp.tile([128, 512], F32, tag="pp", name=f"kps{sc}{h}")
                proj3(ps, sc, DV + h * HD, DV + (h + 1) * HD)
                nc.vector.tensor_scalar_mul(kT[:, h, sc * 512:(sc + 1) * 512], ps, INV_SC)
            for ssl in range(4):
                idx = sc * 4 + ssl
                vps = pp.tile([128, 512], F32, tag="pp", name=f"vps{idx}")
                proj3(vps[:, 0:DV], sc, 2 * DV, 3 * DV,
                      xslice=slice(ssl * 128, (ssl + 1) * 128))
                nc.vector.tensor_scalar_mul(vhi[:, idx, 0:DV], vps[:, 0:DV], INV_SC)
                nc.vector.scalar_tensor_tensor(vlo[:, idx, 0:DV], vps[:, 0:DV], INV_SC,
                                               vhi[:, idx, 0:DV], MULT, SUB)
                nc.vector.tensor_scalar_mul(vaug[:, idx, 0:DV], vps[:, 0:DV], INV_SC)

        # chunk-0 q-projection at the tail of phase 1 (pp pool still open,
        # avoids a cross-pool psum wait at the phase boundary)
        for h in range(2):
            qp = pp.tile([128, 512], F32, tag="pp", name=f"qp0{h}")
            proj3(qp, 0, h * HD, (h + 1) * HD)
            nc.vector.tensor_scalar_mul(qT[:, h, 0:512], qp, INV_SC)
        pctx.close()

        # ---- phase 2: per query chunk: scores, exp, PV ------------------
        ps_sc = ctx.enter_context(tc.tile_pool(name="ps_sc", bufs=2, space=bass.MemorySpace.PSUM))
        ps_pv = ctx.enter_context(tc.tile_pool(name="ps_pv", bufs=4, space=bass.MemorySpace.PSUM))

        def qproj_block(qcn, h):
            # time-shares a score-psum slot (PSUM has no spare bank)
            qp = ps_sc.tile([128, 2, 512], F32, tag="sc", name=f"qp{qcn}{h}")[:, 0, :]
            proj3(qp, qcn, h * HD, (h + 1) * HD)
            nc.vector.tensor_scalar_mul(qT[:, h, qcn * 512:(qcn + 1) * 512], qp, INV_SC)

        def scores(qc, h, g):
            sg = ps_sc.tile([128, 2, 512], F32, tag="sc", name=f"sg{qc}{h}{g}")
            qTs = qT[:, h, qc * 512:(qc + 1) * 512]
            for i in range(2):
                kt = 2 * g + i
                nc.tensor.matmul(sg[:, i, :], kT[:, h, kt * 128:(kt + 1) * 128],
                                 qTs, start=True, stop=True)
            return sg

        def head_loop(qc, h, sg_first):
            pv_t = ps_pv.tile([128, 4, 512], F32, tag="pv", bufs=1, name=f"pv{qc}{h}")
            pvs = [pv_t[:, i, 0:DVA] for i in range(4)]
            sg_prev = sg_first
            for g in range(KG):
                sg_next = scores(qc, h, g + 1) if g + 1 < KG else None
                first, last = (g == 0), (g == KG - 1)
                if h == 0:
                    e8 = e_pool.tile([128, 2, 512], F8, tag="e8", name=f"e8_{qc}{g}")
                    nc.scalar.activation(out=e8, in_=sg_prev,
                                         func=mybir.ActivationFunctionType.Exp,
                                         bias=bias_t, scale=SCALE)
                    for qsl in range(4):
                        stat = e8[:, :, qsl * 128:(qsl + 1) * 128]
                        nc.tensor.matmul(pvs[qsl], stat, vhi[:, 2 * g:2 * g + 2, :],
                                         start=first, stop=False, perf_mode=DR)
                        nc.tensor.matmul(pvs[qsl], stat, vlo[:, 2 * g:2 * g + 2, :],
                                         start=False, stop=last, perf_mode=DR)
                else:
                    eb = e_pool.tile([128, 2, 512], BF16, tag="eb", name=f"eb_{qc}{g}")
                    nc.scalar.activation(out=eb, in_=sg_prev,
                                         func=mybir.ActivationFunctionType.Exp,
                                         bias=bias_t, scale=SCALE)
                    for qsl in range(4):
                        for i in range(2):
                            st = eb[:, i, qsl * 128:(qsl + 1) * 128]
                            nc.tensor.matmul(pvs[qsl], st, vaug[:, 2 * g + i, :],
                                             start=(first and i == 0),
                                             stop=(last and i == 1))
                sg_prev = sg_next

            onum = o_pool.tile([128, 4, DVA], BF16, tag="onum", name=f"on{qc}{h}")
            nc.vector.tensor_copy(onum, pv_t[:, :, 0:DVA])
            nc.sync.dma_start(out=out[h, :, qc * 4:qc * 4 + 4, :], in_=onum)

        for qc in range(QC):
            if qc + 1 < QC and (qc + 1) not in chunks:
                dma_chunk(qc + 1)
            head_loop(qc, 0, scores(qc, 0, 0))
            # next chunk's q-projections, with h2's first score group
            # primed between them so the h2 exp chain starts early
            if qc + 1 < QC:
                qproj_block(qc + 1, 0)
            h2_sg0 = scores(qc, 1, 0)
            if qc + 1 < QC:
                qproj_block(qc + 1, 1)
            head_loop(qc, 1, h2_sg0)

    nc.compile()
    return nc


def _lam(lambda_q1, lambda_q2, lambda_k1, lambda_k2):
    lam_init = 0.8 - 0.6 * math.exp(-0.3 * DEPTH)
    l1 = math.exp(float(np.sum(lambda_q1.astype(np.float64) * lambda_k1.astype(np.float64))))
    l2 = math.exp(float(np.sum(lambda_q2.astype(np.float64) * lambda_k2.astype(np.float64))))
    return l1 + l2 + lam_init


def kernel(x, WQ, WK, WV, lambda_q1, lambda_q2, lambda_k1, lambda_k2):
    if "nc" not in _cache:
        _cache["nc"] = build_nc()
    nc = _cache["nc"]

    f8 = ml_dtypes.float8_e4m3
    lam = _lam(lambda_q1, lambda_q2, lambda_k1, lambda_k2)

    w = np.concatenate([np.asarray(WQ, np.float32), np.asarray(WK, np.float32),
                        np.asarray(WV, np.float32)], axis=1) * W_SC   # [D, 768]
    wh = np.ascontiguousarray(w, dtype=f8)
    wl = np.ascontiguousarray(w - wh.astype(np.float32), dtype=f8)

    in_maps = []
    for c in range(N_CORES):
        b, kh = c // 2, c % 2
        xb = np.asarray(x[b], np.float32)
        if kh:
            xb = np.concatenate([xb[SK:], xb[:SK]], axis=0)
        xbh = np.asarray(xb, f8)
        xbl = np.asarray(xb - xbh.astype(np.float32), f8)
        in_maps.append({
            "xh": np.ascontiguousarray(xbh.T),
            "xl": np.ascontiguousarray(xbl.T),
            "wh": wh, "wl": wl,
        })

    kres = None
    for attempt in range(3):
        try:
            kres = run_bass_kernel_spmd(nc, in_maps, list(range(N_CORES)))
            break
        except (ModuleNotFoundError, ImportError):
            os.environ["BASS_NEVER_TRACE"] = "1"
        except Exception:
            if attempt == 2:
                raise
            time.sleep(5)
    if kres is None:
        kres = run_bass_kernel_spmd(nc, in_maps, list(range(N_CORES)))
    _cache["last_results"] = kres
    _cache["input_names"] = ["xh", "xl", "wh", "wl"]
    res = kres.results

    out = np.empty((B, S, DV), np.float32)
    for b in range(B):
        na = res[2 * b]["out"].astype(np.float32)       # [2, S, DVA], canonical q
        nb = res[2 * b + 1]["out"].astype(np.float32)   # q rotated by SK
        nb = np.concatenate([nb[:, SK:, :], nb[:, :SK, :]], axis=1)
        n = na + nb
        o1 = n[0, :, :DV] / n[0, :, DV:DVA]
        o2 = n[1, :, :DV] / n[1, :, DV:DVA]
        out[b] = o1 - lam * o2
    return out


# revision 12
# speedup vs baseline: 1.3372x; 1.0124x over previous
"""Differential attention kernel for 8 Trainium2 NeuronCores — v5.

Key-sharded: core c handles batch c//2 and key-half c%2 (2048 keys), all
4096 queries. Per-head partial numerators + row sums go back to the host,
which adds the two halves and normalizes (softmax denominators span both
cores' keys).

fp8 (e4m3) DoubleRow matmuls carry the projections and head-1 PV:
- projections use a 3-term hi/lo split (x and W each split into e4m3
  hi + residual lo; terms hi*hi + hi*lo + lo*hi) — bf16-level accuracy at
  0.75x the bf16 matmul cost.
- head-1 PV uses single-e4m3 exp scores against hi/lo-split V — half the
  bf16 PV cost. Head 2 (amplified by lambda≈2.8) stays bf16.
- scores stay bf16 (contraction is only 128, DoubleRow needs 256).
exp is biased by -3 so e4m3's max (224) covers the largest logit (~8.02).

Schedule notes: score emission is software-pipelined one key-group ahead
so the in-order PE queue keeps a score matmul in flight while the
activation engine runs exp. The next chunk's q-projections run between
the two head loops, time-sharing the score-psum slots (PSUM is exactly
full: 4 score banks + 4 PV banks). DMAs are whole-chunk single transfers
(the SP engine serializes DMA dispatch at ~0.4-3us each).
"""

import math
import os
import time
from contextlib import ExitStack

import ml_dtypes
import numpy as np

import concourse.bass as bass
from concourse import bacc
import concourse.mybir as mybir
import concourse.tile as tile
from concourse.bass_utils import run_bass_kernel_spmd

B, S, D = 4, 4096, 2048
HD = 128
DV = 256
DVA = DV + 1      # + ones column for row sums
SK = S // 2       # keys per core
N_CORES = 8
DEPTH = 12
SCALE = HD ** -0.5

DT = D // 128     # 16 d-tiles
SKT = SK // 128   # 16 key tiles per core
KG = SKT // 2     # 8 key groups (DoubleRow pairs)
QC = S // 512     # 8 query chunks
KC = SK // 512    # 4 key chunks (phase-1 k/v projection)

W_SC = 32.0       # W pre-scale before e4m3 (W entries ~N(0, 1/D))
INV_SC = 1.0 / W_SC
ET_BIAS = -3.0    # exp(a - 3): keeps max exp < e4m3 max (224)

F8 = mybir.dt.float8e4
BF16 = mybir.dt.bfloat16
F32 = mybir.dt.float32
DR = mybir.MatmulPerfMode.DoubleRow
MULT = mybir.AluOpType.mult
SUB = mybir.AluOpType.subtract

_cache = {}


def build_nc():
    nc = bacc.Bacc("TRN2", target_bir_lowering=False, debug=False)

    xh_d = nc.declare_dram_parameter("xh", [D, S], F8, isOutput=False)
    xl_d = nc.declare_dram_parameter("xl", [D, S], F8, isOutput=False)
    wh_d = nc.declare_dram_parameter("wh", [D, 3 * DV], F8, isOutput=False)
    wl_d = nc.declare_dram_parameter("wl", [D, 3 * DV], F8, isOutput=False)
    out_d = nc.declare_dram_parameter("out", [2, S, DVA], BF16, isOutput=True)

    # tiled views: [p, dt, col] so one DMA moves a whole chunk (the SP
    # engine serializes DMAs; per-dt dma_starts would swamp it)
    xh = xh_d.ap().rearrange("(dt p) s -> p dt s", p=128)      # [128, DT, S]
    xl = xl_d.ap().rearrange("(dt p) s -> p dt s", p=128)
    wh = wh_d.ap().rearrange("(dt p) c -> p dt c", p=128)      # [128, DT, 768]
    wl = wl_d.ap().rearrange("(dt p) c -> p dt c", p=128)
    out = out_d.ap().rearrange("h (qs p) d -> h p qs d", p=128)  # [2, 128, 32, DVA]

    with tile.TileContext(nc) as tc, ExitStack() as ctx:
        singles = ctx.enter_context(tc.tile_pool(name="singles", bufs=1))
        x_pool = ctx.enter_context(tc.tile_pool(name="x", bufs=10))
        e_pool = ctx.enter_context(tc.tile_pool(name="e", bufs=4))
        o_pool = ctx.enter_context(tc.tile_pool(name="o", bufs=4))

        w_hi = singles.tile([128, DT, 3 * DV], F8, tag="w_hi")
        w_lo = singles.tile([128, DT, 3 * DV], F8, tag="w_lo")
        kT = singles.tile([128, 2, SK], BF16, tag="kT")       # [dh, head, key]
        qT = singles.tile([128, 2, S], BF16, tag="qT")        # [dh, head, query]
        vhi = singles.tile([128, SKT, DVA], F8, tag="vhi")    # [key_row, kt, dv+1]
        vlo = singles.tile([128, SKT, DVA], F8, tag="vlo")
        vaug = singles.tile([128, SKT, DVA], BF16, tag="vaug")
        bias_t = singles.tile([128, 1], F32, tag="bias")

        nc.vector.memset(bias_t, ET_BIAS)
        nc.vector.memset(vhi[:, :, DV:DVA], 1.0)
        nc.vector.memset(vlo[:, :, DV:DVA], 0.0)
        nc.vector.memset(vaug[:, :, DV:DVA], 1.0)

        chunks = {}

        def dma_chunk(sc):
            th = x_pool.tile([128, DT, 512], F8, tag="xt", name=f"xh_{sc}")
            tl = x_pool.tile([128, DT, 512], F8, tag="xt", name=f"xl_{sc}")
            nc.gpsimd.dma_start(out=th, in_=xh[:, :, sc * 512:(sc + 1) * 512])
            nc.gpsimd.dma_start(out=tl, in_=xl[:, :, sc * 512:(sc + 1) * 512])
            chunks[sc] = (th, tl)

        # K weights first, then chunk 0, V weights, chunk 1, Q weights last
        # (q-projection happens in phase 2) — SP runs DMAs strictly in order.
        for c0, c1 in ((DV, 2 * DV),):
            nc.sync.dma_start(out=w_hi[:, :, c0:c1], in_=wh[:, :, c0:c1])
            nc.sync.dma_start(out=w_lo[:, :, c0:c1], in_=wl[:, :, c0:c1])
        dma_chunk(0)
        for c0, c1 in ((2 * DV, 3 * DV),):
            nc.sync.dma_start(out=w_hi[:, :, c0:c1], in_=wh[:, :, c0:c1])
            nc.sync.dma_start(out=w_lo[:, :, c0:c1], in_=wl[:, :, c0:c1])
        dma_chunk(1)

        def proj3(ps, sc, col0, col1, xslice=None):
            """3-term hi/lo split projection: 24 DoubleRow matmuls into ps."""
            th, tl = chunks[sc]
            terms = [(w_hi, th), (w_lo, th), (w_hi, tl)]
            for t, (wt, xt) in enumerate(terms):
                for p in range(DT // 2):
                    first = (t == 0 and p == 0)
                    last = (t == len(terms) - 1 and p == DT // 2 - 1)
                    dpair = slice(2 * p, 2 * p + 2)
                    if xslice is None:
                        nc.tensor.matmul(ps, wt[:, dpair, col0:col1], xt[:, dpair, :],
                                         start=first, stop=last, perf_mode=DR)
                    else:
                        nc.tensor.matmul(ps, xt[:, dpair, xslice], wt[:, dpair, col0:col1],
                                         start=first, stop=last, perf_mode=DR)

        # ---- phase 1: k and v projections over this core's key half -----
        pctx = ExitStack()
        pp = pctx.enter_context(tc.tile_pool(name="pp", bufs=6, space=bass.MemorySpace.PSUM))

        # PE p-state warm-up while initial DMAs land
        jt = singles.tile([128, 512], BF16, tag="junk")
        nc.vector.memset(jt, 0.0)
        jps = pp.tile([128, 512], F32, tag="pp", name="jps")
        for _ in range(14):
            nc.tensor.matmul(jps, jt[:, 0:128], jt, start=True, stop=True)

        for sc in range(KC):
            if sc + 2 < KC:
                dma_chunk(sc + 2)
            elif sc == KC - 2:
                for c0, c1 in ((0, DV),):
                    nc.sync.dma_start(out=w_hi[:, :, c0:c1], in_=wh[:, :, c0:c1])
                    nc.sync.dma_start(out=w_lo[:, :, c0:c1], in_=wl[:, :, c0:c1])
            for h in range(2):
                ps = pp.tile([128, 512], F32, tag="pp", name=f"kps{sc}{h}")
                proj3(ps, sc, DV + h * HD, DV + (h + 1) * HD)
                nc.vector.tensor_scalar_mul(kT[:, h, sc * 512:(sc + 1) * 512], ps, INV_SC)
            for ssl in range(4):
                idx = sc * 4 + ssl
                vps = pp.tile([128, 512], F32, tag="pp", name=f"vps{idx}")
                proj3(vps[:, 0:DV], sc, 2 * DV, 3 * DV,
                      xslice=slice(ssl * 128, (ssl + 1) * 128))
                nc.vector.tensor_scalar_mul(vhi[:, idx, 0:DV], vps[:, 0:DV], INV_SC)
                nc.vector.scalar_tensor_tensor(vlo[:, idx, 0:DV], vps[:, 0:DV], INV_SC,
                                               vhi[:, idx, 0:DV], MULT, SUB)
                nc.vector.tensor_scalar_mul(vaug[:, idx, 0:DV], vps[:, 0:DV], INV_SC)

        # chunk-0 q-projection at the tail of phase 1 (pp pool still open,
        # avoids a cross-pool psum wait at the phase boundary)
        for h in range(2):
            qp = pp.tile([128, 512], F32, tag="pp", name=f"qp0{h}")
            proj3(qp, 0, h * HD, (h + 1) * HD)
            nc.vector.tensor_scalar_mul(qT[:, h, 0:512], qp, INV_SC)
        pctx.close()

        # ---- phase 2: per query chunk: scores, exp, PV ------------------
        ps_sc = ctx.enter_context(tc.tile_pool(name="ps_sc", bufs=2, space=bass.MemorySpace.PSUM))
        ps_pv = ctx.enter_context(tc.tile_pool(name="ps_pv", bufs=4, space=bass.MemorySpace.PSUM))

        def qproj_block(qcn, h):
            # time-shares a score-psum slot (PSUM has no spare bank)
            qp = ps_sc.tile([128, 2, 512], F32, tag="sc", name=f"qp{qcn}{h}")[:, 0, :]
            proj3(qp, qcn, h * HD, (h + 1) * HD)
            nc.vector.tensor_scalar_mul(qT[:, h, qcn * 512:(qcn + 1) * 512], qp, INV_SC)

        def scores(qc, h, g):
            sg = ps_sc.tile([128, 2, 512], F32, tag="sc", name=f"sg{qc}{h}{g}")
            qTs = qT[:, h, qc * 512:(qc + 1) * 512]
            for i in range(2):
                kt = 2 * g + i
                nc.tensor.matmul(sg[:, i, :], kT[:, h, kt * 128:(kt + 1) * 128],
                                 qTs, start=True, stop=True)
            return sg

        def head_loop(qc, h, sg_first, tail_filler=None):
            pv_t = ps_pv.tile([128, 4, 512], F32, tag="pv", bufs=1, name=f"pv{qc}{h}")
            pvs = [pv_t[:, i, 0:DVA] for i in range(4)]
            tail_result = [None]
            sg_prev = sg_first
            for g in range(KG):
                sg_next = scores(qc, h, g + 1) if g + 1 < KG else None
                if g == KG - 1 and tail_filler is not None:
                    # PE filler during the last group's exp wait
                    tail_result[0] = tail_filler()
                first, last = (g == 0), (g == KG - 1)
                if h == 0:
                    e8 = e_pool.tile([128, 2, 512], F8, tag="e8", name=f"e8_{qc}{g}")
                    nc.scalar.activation(out=e8, in_=sg_prev,
                                         func=mybir.ActivationFunctionType.Exp,
                                         bias=bias_t, scale=SCALE)
                    for qsl in range(4):
                        stat = e8[:, :, qsl * 128:(qsl + 1) * 128]
                        nc.tensor.matmul(pvs[qsl], stat, vhi[:, 2 * g:2 * g + 2, :],
                                         start=first, stop=False, perf_mode=DR)
                        nc.tensor.matmul(pvs[qsl], stat, vlo[:, 2 * g:2 * g + 2, :],
                                         start=False, stop=last, perf_mode=DR)
                else:
                    eb = e_pool.tile([128, 2, 512], BF16, tag="eb", name=f"eb_{qc}{g}")
                    nc.scalar.activation(out=eb, in_=sg_prev,
                                         func=mybir.ActivationFunctionType.Exp,
                                         bias=bias_t, scale=SCALE)
                    for qsl in range(4):
                        for i in range(2):
                            st = eb[:, i, qsl * 128:(qsl + 1) * 128]
                            nc.tensor.matmul(pvs[qsl], st, vaug[:, 2 * g + i, :],
                                             start=(first and i == 0),
                                             stop=(last and i == 1))
                sg_prev = sg_next

            onum = o_pool.tile([128, 4, DVA], BF16, tag="onum", name=f"on{qc}{h}")
            nc.vector.tensor_copy(onum, pv_t[:, :, 0:DVA])
            nc.sync.dma_start(out=out[h, :, qc * 4:qc * 4 + 4, :], in_=onum)
            return tail_result[0]

        h1_sg0 = scores(0, 0, 0)
        for qc in range(QC):
            if qc + 1 < QC and (qc + 1) not in chunks:
                dma_chunk(qc + 1)
            # h1's tail filler: next chunk's head-0 q-projection; h2's tail
            # filler: next chunk's first h1 score group (primes its exps)
            head_loop(qc, 0, h1_sg0,
                      tail_filler=(lambda qc=qc: qproj_block(qc + 1, 0))
                      if qc + 1 < QC else None)
            h2_sg0 = scores(qc, 1, 0)
            if qc + 1 < QC:
                qproj_block(qc + 1, 1)
            h1_sg0 = head_loop(qc, 1, h2_sg0,
                               tail_filler=(lambda qc=qc: scores(qc + 1, 0, 0))
                               if qc + 1 < QC else None)

    nc.compile()
    return nc


def _lam(lambda_q1, lambda_q2, lambda_k1, lambda_k2):
    lam_init = 0.8 - 0.6 * math.exp(-0.3 * DEPTH)
    l1 = math.exp(float(np.sum(lambda_q1.astype(np.float64) * lambda_k1.astype(np.float64))))
    l2 = math.exp(float(np.sum(lambda_q2.astype(np.float64) * lambda_k2.astype(np.float64))))
    return l1 + l2 + lam_init


def kernel(x, WQ, WK, WV, lambda_q1, lambda_q2, lambda_k1, lambda_k2):
    if "nc" not in _cache:
        _cache["nc"] = build_nc()
    nc = _cache["nc"]

    f8 = ml_dtypes.float8_e4m3
    lam = _lam(lambda_q1, lambda_q2, lambda_k1, lambda_k2)

    w = np.concatenate([np.asarray(WQ, np.float32), np.asarray(WK, np.float32),
                        np.asarray(WV, np.float32)], axis=1) * W_SC   # [D, 768]
    wh = np.ascontiguousarray(w, dtype=f8)
    wl = np.ascontiguousarray(w - wh.astype(np.float32), dtype=f8)

    in_maps = []
    for c in range(N_CORES):
        b, kh = c // 2, c % 2
        xb = np.asarray(x[b], np.float32)
        if kh:
            xb = np.concatenate([xb[SK:], xb[:SK]], axis=0)
        xbh = np.asarray(xb, f8)
        xbl = np.asarray(xb - xbh.astype(np.float32), f8)
        in_maps.append({
            "xh": np.ascontiguousarray(xbh.T),
            "xl": np.ascontiguousarray(xbl.T),
            "wh": wh, "wl": wl,
        })

    kres = None
    for attempt in range(3):
        try:
            kres = run_bass_kernel_spmd(nc, in_maps, list(range(N_CORES)))
            break
        except (ModuleNotFoundError, ImportError):
            os.environ["BASS_NEVER_TRACE"] = "1"
        except Exception:
            if attempt == 2:
                raise
            time.sleep(5)
    if kres is None:
        kres = run_bass_kernel_spmd(nc, in_maps, list(range(N_CORES)))
    _cache["last_results"] = kres
    _cache["input_names"] = ["xh", "xl", "wh", "wl"]
    res = kres.results

    out = np.empty((B, S, DV), np.float32)
    for b in range(B):
        na = res[2 * b]["out"].astype(np.float32)       # [2, S, DVA], canonical q
        nb = res[2 * b + 1]["out"].astype(np.float32)   # q rotated by SK
        nb = np.concatenate([nb[:, SK:, :], nb[:, :SK, :]], axis=1)
        n = na + nb
        o1 = n[0, :, :DV] / n[0, :, DV:DVA]
        o2 = n[1, :, :DV] / n[1, :, DV:DVA]
        out[b] = o1 - lam * o2
    return out
